# revision 47
# baseline (speedup 1.0000x reference)
"""Trainium2 Bass kernel for causal multi-head attention with RoPE.

Problem shapes (hardcoded): x [2,2048,2048] f32, Wqkv [6144,2048], Wout [2048,2048],
cos/sin [2048,128]. 16 heads x 128 head-dim.

Sharding: tensor-parallel over heads -- 2 heads per core on 8 cores.
Each core computes qkv projection for its heads, RoPE, causal SDPA, and its
slice of the output projection (row-parallel); host sums the 8 partials.

All on-device layouts keep tokens on the free dimension ([dim, tokens]) so no
transposes are ever needed:
  - Q/K produced as qT/kT [hd, tok] directly from the projection.
  - V produced as v [tok, hd] (other matmul orientation).
  - scoresT [k_tok, q_tok] = kT_tile.T @ qT -> softmax over the partition dim:
    no max subtraction (scores are provably bounded ~N(0,1)), l = column sums
    via a ones-vector matmul, normalization applied to the attention output.
  - attention output oT [hd, q_tok] = v_chunk.T @ probsT, accumulated in PSUM.
  - output projection outT [D, tok] = WoutT_chunk.T @ oT.
RoPE rotate_half is a fixed +-1 permutation => done with a 128x128 matmul.
"""

import numpy as np
import ml_dtypes

B, T, D, H = 2, 2048, 2048, 16
HD = 128
N_CORES = 8
HPC = H // N_CORES          # heads per core = 2
TOK = B * T                 # 4096 flattened tokens
NT = TOK // 512             # 8 token tiles of 512
KC = D // 128               # 16 contraction chunks for the projections
SCALE = 1.0 / float(np.sqrt(HD))

BF16 = ml_dtypes.bfloat16

_CACHE = {}


def _emit_body(nc, tc, pools, io, use_pool=False, out_bf16=True,
               dma_rot=True, spread2=False, osplit=2, oq=2,
               lmode="mm", lacc="dve", vmode="x", trim=True,
               pmode="single", smode="qbo", rbf16=False,
               p2p=False, p2s=False, p2o=False,
               mmask=False, oeng=None, vorder="last", pre0=4,
               early_l=False, peng="aa"):
    """Emit one full forward pass, batch-pipelined."""
    import concourse.bass as bass  # noqa: F401
    import concourse.mybir as mybir

    dt = mybir.dt
    (consts, xpool, big, temps, ntemps, probsp, outp, accp, psum, psum2,
     psum_o, psum_l) = pools
    (xt_d, wq_d, wk_d, wv_d, wo_d, cs_d, sn_d, sns_d, pt_d, onec_d, oner_d,
     msk_d, mskt_d, id_d, out_d) = io

    # ---- resident constants / weights in SBUF ----
    # startup order matters: the first proj matmul needs (wqa chunk 0, xt
    # chunk 0) — emit those DMAs first in small pieces so it can issue early;
    # cos/sin aren't read until the first rope epilogue ~20us later.
    wq_r = wq_d.rearrange("(c p) e -> p c e", p=128)
    wqa_sb = consts.tile([128, KC // 2, 256], dt.bfloat16, tag="wqa")
    wqb_sb = consts.tile([128, KC // 2, 256], dt.bfloat16, tag="wqb")
    wk_sb = consts.tile([128, KC, 256], dt.bfloat16, tag="wk")
    wv_sb = consts.tile([128, KC, 256], dt.bfloat16, tag="wv")
    nc.sync.dma_start(out=wqa_sb[:, 0:2, :], in_=wq_r[:, 0:2, :])
    cs_sb = consts.tile([128, T], dt.bfloat16, tag="cs")
    sn_sb = consts.tile([128, T], dt.bfloat16, tag="sn")
    if not dma_rot:
        pt_sb = consts.tile([128, 128], dt.bfloat16, tag="pt")
        nc.sync.dma_start(out=pt_sb, in_=pt_d[:])
    if lmode == "mm":
        onec_sb = consts.tile([128, 1], dt.bfloat16, tag="onec")
        nc.sync.dma_start(out=onec_sb, in_=onec_d[:])
        oner_sb = consts.tile([1, 128], dt.bfloat16, tag="oner")
        nc.sync.dma_start(out=oner_sb, in_=oner_d[:])
    if lmode == "mm1":
        # all-ones [128,128]: lhsT for the column-sum+broadcast matmul
        ones_sb = consts.tile([128, 128], dt.bfloat16, tag="ones")
        nc.vector.memset(ones_sb, 1.0)
    wo_sb = consts.tile([128, HPC, D], dt.bfloat16, tag="wo")
    msk_sb = (consts.tile([128, 4, 512], dt.bfloat16, tag="msk", name="msk")
              if not mmask else None)
    if mmask:
        # causal mask as PE psum-accumulate: mskt = (step mask).T, id = I128
        mskt_sb = consts.tile([128, 128], dt.bfloat16, tag="mskt")
        id_sb = consts.tile([128, 128], dt.bfloat16, tag="id")

    # ---- resident activations: per-(head,batch) for fine-grained deps ----
    qTs = {(h, b): big.tile([128, T], dt.bfloat16, tag=f"qT{h}{b}", name=f"qT{h}{b}")
           for h in range(HPC) for b in range(B)}
    kTs = {(h, b): big.tile([128, T], dt.bfloat16, tag=f"kT{h}{b}", name=f"kT{h}{b}")
           for h in range(HPC) for b in range(B)}
    oTs = {(h, b): big.tile([128, T], dt.bfloat16, tag=f"oT{h}{b}", name=f"oT{h}{b}")
           for h in range(HPC) for b in range(B)}
    vss = {b: big.tile([128, 16, 256], dt.bfloat16, tag=f"v{b}", name=f"v{b}")
           for b in range(B)}

    xt_r = xt_d.rearrange("(c p) t -> p c t", p=128)  # [128, 16, 4096]
    xt_first = []
    for xh in range(2):
        xt_sb = xpool.tile([128, KC // 2, 512], dt.bfloat16, tag=f"xt{xh}",
                           name=f"xtp{xh}")
        if xh == 0:
            # split: chunk 0 lands first so the first matmul can start
            nc.sync.dma_start(out=xt_sb[:, 0:2, :], in_=xt_r[:, 0:2, 0:512])
            nc.sync.dma_start(out=xt_sb[:, 2:, :], in_=xt_r[:, 2:8, 0:512])
        else:
            nc.sync.dma_start(out=xt_sb, in_=xt_r[:, xh * 8:(xh + 1) * 8, 0:512])
        xt_first.append(xt_sb)
    nc.sync.dma_start(out=wqa_sb[:, 2:, :], in_=wq_r[:, 2:KC // 2, :])

    def late_consts():
        nc.sync.dma_start(out=wqb_sb, in_=wq_r[:, KC // 2:, :])
        nc.sync.dma_start(out=cs_sb, in_=cs_d[:])
        nc.sync.dma_start(out=sn_sb, in_=(sns_d[:] if dma_rot else sn_d[:]))
        if vorder == "smart":
            # V projection runs before K: load wv ahead of wk
            nc.sync.dma_start(out=wv_sb,
                              in_=wv_d.rearrange("(c p) e -> p c e", p=128))
            nc.sync.dma_start(out=wk_sb,
                              in_=wk_d.rearrange("(c p) e -> p c e", p=128))
        else:
            nc.sync.dma_start(out=wk_sb,
                              in_=wk_d.rearrange("(c p) e -> p c e", p=128))
            nc.sync.dma_start(out=wv_sb,
                              in_=wv_d.rearrange("(c p) e -> p c e", p=128))

    def _load_xts(t0g, split_first=False):
        xts = []
        for xh in range(2):
            xt_sb = xpool.tile([128, KC // 2, 512], dt.bfloat16,
                               tag=f"xt{xh}", name=f"xt{xh}")
            if xh == 0 and split_first:
                nc.sync.dma_start(out=xt_sb[:, 0:2, :],
                                  in_=xt_r[:, 0:2, t0g:t0g + 512])
                nc.sync.dma_start(out=xt_sb[:, 2:, :],
                                  in_=xt_r[:, 2:8, t0g:t0g + 512])
            else:
                nc.sync.dma_start(
                    out=xt_sb, in_=xt_r[:, xh * 8:(xh + 1) * 8, t0g:t0g + 512])
            xts.append(xt_sb)
        return xts

    def proj_pair(b, tp, gp=2, vpos="last"):
        """Project gp adjacent 512-token tiles; the kc loop issues the tiles'
        matmuls back-to-back so each weight chunk is loaded once. vpos places
        the V projection first/mid/last among the three so its DMA transposes
        into vss finish before SDPA's o-matmuls need them."""
        tls = tuple(tp * gp + j for j in range(gp))
        t0s = [ttl * 512 for ttl in tls]
        if b == 0 and tp == 0:
            xtss = [xt_first] + [_load_xts(512 * ttl, split_first=True)
                                 for ttl in tls[1:]]
            late_consts()
        else:
            xtss = [_load_xts((b * 4 + ttl) * 512) for ttl in tls]

        def epilogue(ps, dsts, m, t0, raw_in=False, ceng="a"):
            if raw_in:
                raw = ps
            else:
                raw = temps.tile([128, 512], dt.bfloat16, tag="raw")
                if ceng == "v":
                    nc.vector.tensor_copy(out=raw, in_=ps)
                else:
                    nc.scalar.copy(out=raw, in_=ps)
            rsb = temps.tile([128, 512], dt.bfloat16, tag="rsb")
            nc.sync.dma_start(out=rsb[0:64, :], in_=raw[64:128, :])
            nc.sync.dma_start(out=rsb[64:128, :], in_=raw[0:64, :])
            t1 = temps.tile([128, 512], dt.bfloat16, tag="t1")
            nc.vector.tensor_tensor(t1, raw, cs_sb[:, t0:t0 + 512],
                                    mybir.AluOpType.mult)
            t2 = temps.tile([128, 512], dt.bfloat16, tag="t2")
            nc.vector.tensor_tensor(t2, rsb, sn_sb[:, t0:t0 + 512],
                                    mybir.AluOpType.mult)
            eng = nc.gpsimd if use_pool else nc.vector
            eng.tensor_tensor(dsts[(m, b)][:, t0:t0 + 512], t1, t2,
                              mybir.AluOpType.add)

        def alloc_ps(nm):
            if p2p:
                t2 = psum2.tile([128, 2, 512], dt.float32, tag="s2", name=nm)
                return [t2[:, i, :] for i in range(2)], t2
            return [psum.tile([128, 512], dt.float32, tag="s", name=f"{nm}{i}")
                    for i in range(gp)], None

        def qk_phase(w_sb, dsts, ceng="a"):
            for m in range(HPC):
                pss, ps2 = alloc_ps("psp")
                for kc in range(KC):
                    if isinstance(w_sb, tuple):
                        w_ap = (w_sb[1] if kc < 8 else w_sb[2])[:, kc % 8,
                                                               m * 128:(m + 1) * 128]
                    else:
                        w_ap = w_sb[:, kc, m * 128:(m + 1) * 128]
                    for i in range(gp):
                        nc.tensor.matmul(
                            pss[i], lhsT=w_ap,
                            rhs=xtss[i][kc // 8][:, kc % 8, :],
                            start=(kc == 0), stop=(kc == KC - 1),
                            skip_group_check=(i > 0 and p2p),
                        )
                if p2p:
                    raw2 = temps.tile([128, 2, 512], dt.bfloat16, tag="raw2")
                    nc.scalar.copy(out=raw2, in_=ps2)
                    for i in range(gp):
                        epilogue(raw2[:, i, :], dsts, m, t0s[i], raw_in=True)
                else:
                    for i in range(gp):
                        epilogue(pss[i], dsts, m, t0s[i], ceng=ceng)

        def v_phase():
            for m in range(HPC):
                pss, ps2 = alloc_ps("psv")
                for kc in range(KC):
                    w_ap = wv_sb[:, kc, m * 128:(m + 1) * 128]
                    for i in range(gp):
                        nc.tensor.matmul(
                            pss[i], lhsT=w_ap,
                            rhs=xtss[i][kc // 8][:, kc % 8, :],
                            start=(kc == 0), stop=(kc == KC - 1),
                            skip_group_check=(i > 0 and p2p),
                        )
                if p2p:
                    vtmp2 = temps.tile([128, 2, 512], dt.bfloat16, tag="raw2")
                    nc.scalar.copy(out=vtmp2, in_=ps2)
                    for i in range(gp):
                        for tcc in range(4):
                            nc.sync.dma_start_transpose(
                                out=vss[b][:, tls[i] * 4 + tcc,
                                           m * 128:(m + 1) * 128],
                                in_=vtmp2[:, i, tcc * 128:(tcc + 1) * 128])
                else:
                    for i in range(gp):
                        vtmp = temps.tile([128, 512], dt.bfloat16, tag="raw")
                        nc.scalar.copy(out=vtmp, in_=pss[i])
                        for tcc in range(4):
                            nc.sync.dma_start_transpose(
                                out=vss[b][:, tls[i] * 4 + tcc,
                                           m * 128:(m + 1) * 128],
                                in_=vtmp[:, tcc * 128:(tcc + 1) * 128])

        qph = [(("wqsplit", wqa_sb, wqb_sb), qTs, peng[0]),
               (wk_sb, kTs, peng[1])]
        if vpos == "first":
            v_phase()
        for pi, (w_sb, dsts, ce) in enumerate(qph):
            qk_phase(w_sb, dsts, ceng=ce)
            if pi == 0 and vpos == "mid":
                v_phase()
        if vpos == "last":
            v_phase()

    def proj_tile(b, ttl):
        t0g = (b * 4 + ttl) * 512   # global token offset
        t0 = ttl * 512              # within-batch offset
        if b == 0 and ttl == 0:
            xts = xt_first
            late_consts()
        else:
            xts = _load_xts(t0g)

        for w_sb, dsts in ((("wqsplit", wqa_sb, wqb_sb), qTs), (wk_sb, kTs)):
            for m in range(HPC):
                ps = psum.tile([128, 512], dt.float32, tag="s")
                for kc in range(KC):
                    if isinstance(w_sb, tuple):
                        w_ap = (w_sb[1] if kc < 8 else w_sb[2])[:, kc % 8,
                                                               m * 128:(m + 1) * 128]
                    else:
                        w_ap = w_sb[:, kc, m * 128:(m + 1) * 128]
                    nc.tensor.matmul(
                        ps,
                        lhsT=w_ap,
                        rhs=xts[kc // 8][:, kc % 8, :],
                        start=(kc == 0), stop=(kc == KC - 1),
                    )
                raw = temps.tile([128, 512], dt.bfloat16, tag="raw")
                nc.scalar.copy(out=raw, in_=ps)
                if dma_rot:
                    rsb = temps.tile([128, 512], dt.bfloat16, tag="rsb")
                    nc.sync.dma_start(out=rsb[0:64, :], in_=raw[64:128, :])
                    nc.sync.dma_start(out=rsb[64:128, :], in_=raw[0:64, :])
                else:
                    psr = psum.tile([128, 512], dt.float32, tag="s")
                    nc.tensor.matmul(psr, lhsT=pt_sb, rhs=raw, start=True,
                                     stop=True)
                    rsb = temps.tile([128, 512], dt.bfloat16, tag="rsb")
                    nc.scalar.copy(out=rsb, in_=psr)
                t1 = temps.tile([128, 512], dt.bfloat16, tag="t1")
                nc.vector.tensor_tensor(t1, raw, cs_sb[:, t0:t0 + 512],
                                        mybir.AluOpType.mult)
                t2 = temps.tile([128, 512], dt.bfloat16, tag="t2")
                nc.vector.tensor_tensor(t2, rsb, sn_sb[:, t0:t0 + 512],
                                        mybir.AluOpType.mult)
                eng = nc.gpsimd if use_pool else nc.vector
                eng.tensor_tensor(dsts[(m, b)][:, t0:t0 + 512], t1, t2,
                                  mybir.AluOpType.add)

        if vmode == "dmat":
            # V as vT [hd, tok] (512-free matmuls), then DMA-transpose into
            # the [tok, hd] layout the o-matmul needs.
            for m in range(HPC):
                psv = psum.tile([128, 512], dt.float32, tag="s")
                for kc in range(KC):
                    nc.tensor.matmul(
                        psv,
                        lhsT=wv_sb[:, kc, m * 128:(m + 1) * 128],
                        rhs=xts[kc // 8][:, kc % 8, :],
                        start=(kc == 0), stop=(kc == KC - 1),
                    )
                vtmp = temps.tile([128, 512], dt.bfloat16, tag="raw")
                nc.scalar.copy(out=vtmp, in_=psv)
                for tcc in range(4):
                    nc.sync.dma_start_transpose(
                        out=vss[b][:, ttl * 4 + tcc, m * 128:(m + 1) * 128],
                        in_=vtmp[:, tcc * 128:(tcc + 1) * 128])
        else:
            # V -> [tok, hd]; two 256-wide groups share one psum bank
            for pair in range(2):
                psv = psum.tile([128, 512], dt.float32, tag="s")
                for half in range(2):
                    sub = pair * 2 + half
                    for kc in range(KC):
                        nc.tensor.matmul(
                            psv[:, half * 256:(half + 1) * 256],
                            lhsT=xts[kc // 8][:, kc % 8, sub * 128:(sub + 1) * 128],
                            rhs=wv_sb[:, kc, :],
                            start=(kc == 0 and half == 0),
                            stop=(kc == KC - 1),
                            skip_group_check=(half == 1),
                        )
                nc.scalar.copy(
                    out=vss[b][:, ttl * 4 + pair * 2: ttl * 4 + pair * 2 + 2, :],
                    in_=psv)

    def kco_pass(b, h, adv, on_done=None):
        """SDPA for all 4 q-blocks of (b,h), k-chunk-outer: the 4 scores
        matmuls share one kT weight load, the 4 o-matmuls share one V load.
        adv(n) advances the woven outproj generator queue."""
        two = lacc in ("dve2", "pd2")
        ps_os = {qb: psum_o.tile([128, 512], dt.float32, tag="acc_o",
                                 name=f"pso{qb}") for qb in range(4)}
        if two:
            accs = {qb: accp.tile([128, 2, 512], dt.bfloat16, tag="acc",
                                  name=f"acc{qb}") for qb in range(4)}
            aengs = ((nc.vector, nc.vector) if lacc == "dve2"
                     else (nc.gpsimd, nc.vector))
        else:
            accs = {qb: accp.tile([128, 512], dt.bfloat16, tag="acc",
                                  name=f"acc{qb}") for qb in range(4)}
            aeng = nc.gpsimd if lacc == "pool" else nc.vector
        for kc in range(16):
            qlo = kc // 4
            prs = {}
            for qb in range(qlo, 4):
                moff = kc - 4 * qb
                qs = max(moff, 0) * 128 if trim else 0
                q0 = qb * 512
                diag = mmask and moff >= 0
                ps_s = psum.tile([128, 512], dt.float32, tag="s",
                                 name="ps_s")
                nc.tensor.matmul(
                    ps_s[:, qs:],
                    lhsT=kTs[(h, b)][:, kc * 128:(kc + 1) * 128],
                    rhs=qTs[(h, b)][:, q0 + qs:q0 + 512],
                    start=True, stop=not diag,
                )
                if diag:
                    ms = moff * 128
                    nc.tensor.matmul(
                        ps_s[:, ms:ms + 128], lhsT=mskt_sb, rhs=id_sb,
                        start=False, stop=True, skip_group_check=True,
                    )
                pr = probsp.tile([128, 512], dt.bfloat16, tag="probs")
                nc.scalar.activation(pr[:, qs:], ps_s[:, qs:],
                                     mybir.ActivationFunctionType.Exp,
                                     scale=SCALE)
                if moff >= 0 and not mmask:
                    eng = nc.gpsimd if use_pool else nc.vector
                    eng.tensor_tensor(pr[:, qs:qs + 128], pr[:, qs:qs + 128],
                                      msk_sb[:, moff, qs:qs + 128],
                                      mybir.AluOpType.mult)
                if two:
                    c = kc % 2
                    if kc < 2:
                        aengs[c].tensor_copy(out=accs[qb][:, c, qs:],
                                             in_=pr[:, qs:])
                    else:
                        aengs[c].tensor_tensor(accs[qb][:, c, qs:],
                                               accs[qb][:, c, qs:],
                                               pr[:, qs:],
                                               mybir.AluOpType.add)
                elif kc == 0:
                    aeng.tensor_copy(out=accs[qb], in_=pr)
                else:
                    aeng.tensor_tensor(accs[qb][:, qs:], accs[qb][:, qs:],
                                       pr[:, qs:], mybir.AluOpType.add)
                prs[qb] = (pr, qs)
            for qb in range(qlo, 4):
                pr, qs = prs[qb]
                nc.tensor.matmul(
                    ps_os[qb][:, qs:],
                    lhsT=vss[b][:, kc, h * 128:(h + 1) * 128],
                    rhs=pr[:, qs:],
                    start=(kc == 0), stop=(kc == 4 * qb + 3),
                    skip_group_check=(qs > 0),
                )
            if kc % 4 == 3:
                # q-block qlo is complete: normalize it
                qb = qlo
                ps_b = psum.tile([128, 512], dt.float32, tag="s",
                                 name="ps_b")
                if two:
                    qc1 = 128 if qb == 0 else 0
                    nc.tensor.matmul(ps_b, lhsT=ones_sb, rhs=accs[qb][:, 0, :],
                                     start=True, stop=False)
                    nc.tensor.matmul(ps_b[:, qc1:], lhsT=ones_sb,
                                     rhs=accs[qb][:, 1, qc1:],
                                     start=False, stop=True,
                                     skip_group_check=True)
                else:
                    nc.tensor.matmul(ps_b, lhsT=ones_sb, rhs=accs[qb],
                                     start=True, stop=True)
                rb = ntemps.tile([128, 512], dt.float32, tag="rb")
                nc.vector.reciprocal(rb, ps_b)
                nc.vector.tensor_tensor(oTs[(h, b)][:, qb * 512:(qb + 1) * 512],
                                        ps_os[qb], rb, mybir.AluOpType.mult)
                if on_done is not None:
                    on_done(qb)
            adv(2 if h == 1 else 1)

    def sdpa_block(b, qb, h, inter=None, per_kc=0.0):
        q0 = qb * 512
        nk = 4 * (qb + 1)
        steps = 0.0
        two = lacc in ("dve2", "pd2", "pp2")
        ps_b_early = None
        ps_o = psum_o.tile([128, 512], dt.float32, tag="acc_o")
        if lmode == "mm":
            ps_l = psum_l.tile([1, 512], dt.float32, tag="acc_l")
        elif two:
            acc = accp.tile([128, 2, 512], dt.bfloat16, tag="acc")
            aengs = {"dve2": (nc.vector, nc.vector),
                     "pd2": (nc.gpsimd, nc.vector),
                     "pp2": (nc.gpsimd, nc.gpsimd)}[lacc]
            first_qs = {}
        else:
            acc = accp.tile([128, 512], dt.bfloat16, tag="acc")
            if lacc == "mixed":
                aeng = nc.vector if qb >= 2 else nc.gpsimd
            else:
                aeng = nc.gpsimd if lacc == "pool" else nc.vector
        def qs_of(kc):
            moff = kc - 4 * qb
            return max(moff, 0) * 128 if trim else 0, moff

        def post_exp(kc, j, pr_ap):
            """mask + l-accumulate + o-matmul for one kc chunk."""
            nonlocal steps, ps_b_early
            qs, moff = qs_of(kc)
            if moff >= 0 and not mmask:
                eng = nc.gpsimd if use_pool else nc.vector
                eng.tensor_tensor(pr_ap[:, qs:qs + 128], pr_ap[:, qs:qs + 128],
                                  msk_sb[:, moff, qs:qs + 128],
                                  mybir.AluOpType.mult)
            if two:
                c = kc % 2
                if kc < 2:
                    first_qs[c] = qs
                    aengs[c].tensor_copy(out=acc[:, c, qs:], in_=pr_ap[:, qs:])
                else:
                    aengs[c].tensor_tensor(acc[:, c, qs:], acc[:, c, qs:],
                                           pr_ap[:, qs:], mybir.AluOpType.add)
            elif kc == 0:
                aeng.tensor_copy(out=acc, in_=pr_ap)
            else:
                aeng.tensor_tensor(acc[:, qs:], acc[:, qs:], pr_ap[:, qs:],
                                   mybir.AluOpType.add)
            if two and early_l and lmode != "mm":
                # chain c's last add lands at kc = nk-2+c: issue its l-sum
                # matmul immediately so the block-end ps_b wait shrinks
                if kc == nk - 2:
                    ps_b_early = psum.tile([128, 512], dt.float32, tag="s",
                                           name="ps_be")
                    nc.tensor.matmul(ps_b_early, lhsT=ones_sb,
                                     rhs=acc[:, 0, :], start=True, stop=False)
                elif kc == nk - 1:
                    qc1 = first_qs[1]
                    nc.tensor.matmul(ps_b_early[:, qc1:], lhsT=ones_sb,
                                     rhs=acc[:, 1, qc1:], start=False,
                                     stop=True, skip_group_check=True)
            nc.tensor.matmul(
                ps_o[:, qs:],
                lhsT=vss[b][:, kc, h * 128:(h + 1) * 128],
                rhs=pr_ap[:, qs:],
                start=(kc == 0), stop=(kc == nk - 1),
                skip_group_check=(qs > 0),
            )
            if inter is not None:
                steps += per_kc
                while steps >= 1.0:
                    next(inter, None)
                    steps -= 1.0

        def scores_mm(ps_ap, kc, qs, moff, skip_gc=False):
            """scores matmul (+ causal bias via PE accumulate when mmask)."""
            diag = mmask and moff >= 0
            nc.tensor.matmul(
                ps_ap[:, qs:],
                lhsT=kTs[(h, b)][:, kc * 128:(kc + 1) * 128],
                rhs=qTs[(h, b)][:, q0 + qs:q0 + 512],
                start=True, stop=not diag,
                skip_group_check=skip_gc,
            )
            if diag:
                ms = moff * 128
                nc.tensor.matmul(
                    ps_ap[:, ms:ms + 128], lhsT=mskt_sb, rhs=id_sb,
                    start=False, stop=True, skip_group_check=True,
                )

        if p2s:
            for kp in range(nk // 2):
                kcs = (2 * kp, 2 * kp + 1)
                ps2 = psum2.tile([128, 2, 512], dt.float32, tag="s2",
                                 name="ps_s2")
                qss = []
                for j, kc in enumerate(kcs):
                    qs, moff = qs_of(kc)
                    qss.append(qs)
                    scores_mm(ps2[:, j, :], kc, qs, moff, skip_gc=(j > 0))
                qsp = min(qss)
                pr2 = probsp.tile([128, 2, 512], dt.bfloat16, tag="probs")
                nc.scalar.activation(pr2[:, :, qsp:], ps2[:, :, qsp:],
                                     mybir.ActivationFunctionType.Exp,
                                     scale=SCALE)
                for j, kc in enumerate(kcs):
                    post_exp(kc, j, pr2[:, j, :])
        else:
          for kc in range(nk):
            qs, moff = qs_of(kc)
            ps_s = psum.tile([128, 512], dt.float32, tag="s")
            scores_mm(ps_s, kc, qs, moff)
            pr = probsp.tile([128, 512], dt.bfloat16, tag="probs")
            nc.scalar.activation(pr[:, qs:], ps_s[:, qs:],
                                 mybir.ActivationFunctionType.Exp,
                                 scale=SCALE)
            if lmode == "mm":
                if moff >= 0 and not mmask:
                    eng = nc.gpsimd if use_pool else nc.vector
                    eng.tensor_tensor(pr[:, qs:qs + 128], pr[:, qs:qs + 128],
                                      msk_sb[:, moff, qs:qs + 128],
                                      mybir.AluOpType.mult)
                nc.tensor.matmul(ps_l[:, qs:], lhsT=onec_sb, rhs=pr[:, qs:],
                                 start=(kc == 0), stop=(kc == nk - 1),
                                 skip_group_check=(qs > 0))
                nc.tensor.matmul(
                    ps_o[:, qs:],
                    lhsT=vss[b][:, kc, h * 128:(h + 1) * 128],
                    rhs=pr[:, qs:],
                    start=(kc == 0), stop=(kc == nk - 1),
                    skip_group_check=(qs > 0),
                )
                if inter is not None:
                    steps += per_kc
                    while steps >= 1.0:
                        next(inter, None)
                        steps -= 1.0
            else:
                post_exp(kc, 0, pr)
        # normalize: oT = ps_o * recip(broadcast(l))
        if lmode == "mm":
            l_row = ntemps.tile([1, 512], dt.bfloat16, tag="l_row")  # bf16 keeps bcast mm fast
            nc.scalar.copy(out=l_row, in_=ps_l)
            ps_b = psum.tile([128, 512], dt.float32, tag="s")
            nc.tensor.matmul(ps_b, lhsT=oner_sb, rhs=l_row, start=True,
                             stop=True)
        else:
            if two and early_l and ps_b_early is not None:
                ps_b = ps_b_early
            else:
                if p2p and p2s and p2o:
                    ps_b2 = psum2.tile([128, 2, 512], dt.float32, tag="s2",
                                       name="ps_b2")
                    ps_b = ps_b2[:, 0, :]
                else:
                    ps_b = psum.tile([128, 512], dt.float32, tag="s")
                if two:
                    for c in (0, 1):
                        qc = first_qs[c]
                        nc.tensor.matmul(ps_b[:, qc:], lhsT=ones_sb,
                                         rhs=acc[:, c, qc:],
                                         start=(c == 0), stop=(c == 1),
                                         skip_group_check=(c == 1))
                else:
                    nc.tensor.matmul(ps_b, lhsT=ones_sb, rhs=acc, start=True,
                                     stop=True)
        rb = ntemps.tile([128, 512],
                         dt.bfloat16 if rbf16 else dt.float32, tag="rb")
        if rbf16:
            with nc.allow_low_precision(reason="softmax denom bf16"):
                nc.vector.reciprocal(rb, ps_b)
        else:
            nc.vector.reciprocal(rb, ps_b)
        nc.vector.tensor_tensor(oTs[(h, b)][:, q0:q0 + 512], ps_o, rb,
                                mybir.AluOpType.mult)

    out_r = out_d.rearrange("(g p) t -> p g t", p=128)

    def ocopy(dtile, o_ap, ps_ap):
        if oeng is not None:
            ch = oeng[dtile % len(oeng)]
            if ch == "a":
                nc.scalar.copy(out=o_ap, in_=ps_ap)
            elif ch == "v":
                nc.vector.tensor_copy(out=o_ap, in_=ps_ap)
            else:
                nc.gpsimd.tensor_copy(out=o_ap, in_=ps_ap)
        elif dtile % osplit == 0:
            nc.scalar.copy(out=o_ap, in_=ps_ap)
        else:
            nc.vector.tensor_copy(out=o_ap, in_=ps_ap)

    def outproj_gen(b, qb):
        q0 = qb * 512
        t0g = b * T + q0
        if p2o:
            for dp in range(D // 256):   # pairs of 128-wide d tiles
                o_sb = outp.tile([128, 2, 512],
                                 dt.bfloat16 if out_bf16 else dt.float32,
                                 tag="o_sb", name="o_sb")
                ps2 = psum2.tile([128, 2, 512], dt.float32, tag="s2",
                                 name="ps_op2")
                for half in range(2):
                    dtile = dp * 2 + half
                    for h in range(HPC):
                        nc.tensor.matmul(
                            ps2[:, half, :],
                            lhsT=wo_sb[:, h, dtile * 128:(dtile + 1) * 128],
                            rhs=oTs[(h, b)][:, q0:q0 + 512],
                            start=(h == 0), stop=(h == HPC - 1),
                            skip_group_check=(half == 1),
                        )
                ocopy(dp, o_sb, ps2)
                nc.sync.dma_start(
                    out=out_r[:, dp * 2:(dp + 1) * 2, t0g:t0g + 512], in_=o_sb)
                yield
            return
        for dp in range(D // (128 * oq)):   # groups of oq 128-wide d tiles
            o_sb = outp.tile([128, oq, 512],
                             dt.bfloat16 if out_bf16 else dt.float32,
                             tag="o_sb", name="o_sb")
            for half in range(oq):
                dtile = dp * oq + half
                ps = psum.tile([128, 512], dt.float32, tag="s", name="ps_op")
                for h in range(HPC):
                    nc.tensor.matmul(
                        ps,
                        lhsT=wo_sb[:, h, dtile * 128:(dtile + 1) * 128],
                        rhs=oTs[(h, b)][:, q0:q0 + 512],
                        start=(h == 0), stop=(h == HPC - 1),
                    )
                ocopy(dtile, o_sb[:, half, :], ps)
            nc.sync.dma_start(
                out=out_r[:, dp * oq:(dp + 1) * oq, t0g:t0g + 512], in_=o_sb)
            yield

    def sdpa_dual(b, qb, inter=None, pre=0):
        """Both heads' SDPA interleaved kc-step by kc-step for one q-block.
        Outproj steps are pre-drained (covering the wait on K/V readiness at
        block 0) then woven between kc steps."""
        q0 = qb * 512
        nk = 4 * (qb + 1)
        if inter is not None:
            for _ in range(pre):
                next(inter, None)
        steps = 0.0
        per_kc = max(16.0 / oq - pre, 0) / nk
        ps_os = {h: psum_o.tile([128, 512], dt.float32, tag="acc_o",
                                name=f"pso{h}") for h in (0, 1)}
        accs = {h: accp.tile([128, 2, 512], dt.bfloat16, tag="acc",
                             name=f"accd{h}") for h in (0, 1)}
        aengs = ((nc.gpsimd, nc.vector) if lacc == "pd2"
                 else (nc.vector, nc.vector))
        for kc in range(nk):
            moff = kc - 4 * qb
            qs = max(moff, 0) * 128 if trim else 0
            diag = mmask and moff >= 0
            prs = {}
            for h in (0, 1):
                ps_s = psum.tile([128, 512], dt.float32, tag="s",
                                 name=f"ps_s{h}")
                nc.tensor.matmul(
                    ps_s[:, qs:],
                    lhsT=kTs[(h, b)][:, kc * 128:(kc + 1) * 128],
                    rhs=qTs[(h, b)][:, q0 + qs:q0 + 512],
                    start=True, stop=not diag,
                )
                if diag:
                    ms = moff * 128
                    nc.tensor.matmul(ps_s[:, ms:ms + 128], lhsT=mskt_sb,
                                     rhs=id_sb, start=False, stop=True,
                                     skip_group_check=True)
                pr = probsp.tile([128, 512], dt.bfloat16, tag="probs",
                                 name=f"pr{h}")
                nc.scalar.activation(pr[:, qs:], ps_s[:, qs:],
                                     mybir.ActivationFunctionType.Exp,
                                     scale=SCALE)
                if not mmask and moff >= 0:
                    eng = nc.gpsimd if use_pool else nc.vector
                    eng.tensor_tensor(pr[:, qs:qs + 128], pr[:, qs:qs + 128],
                                      msk_sb[:, moff, qs:qs + 128],
                                      mybir.AluOpType.mult)
                c = kc % 2
                if kc < 2:
                    aengs[c].tensor_copy(out=accs[h][:, c, qs:],
                                         in_=pr[:, qs:])
                else:
                    aengs[c].tensor_tensor(accs[h][:, c, qs:],
                                           accs[h][:, c, qs:],
                                           pr[:, qs:], mybir.AluOpType.add)
                prs[h] = pr
            for h in (0, 1):
                nc.tensor.matmul(
                    ps_os[h][:, qs:],
                    lhsT=vss[b][:, kc, h * 128:(h + 1) * 128],
                    rhs=prs[h][:, qs:],
                    start=(kc == 0), stop=(kc == nk - 1),
                    skip_group_check=(qs > 0),
                )
            if inter is not None:
                steps += per_kc
                while steps >= 1.0:
                    next(inter, None)
                    steps -= 1.0
        for h in (0, 1):
            ps_b = psum.tile([128, 512], dt.float32, tag="s", name=f"ps_b{h}")
            qc1 = 128 if qb == 0 else 0
            nc.tensor.matmul(ps_b, lhsT=ones_sb, rhs=accs[h][:, 0, :],
                             start=True, stop=False)
            nc.tensor.matmul(ps_b[:, qc1:], lhsT=ones_sb,
                             rhs=accs[h][:, 1, qc1:], start=False, stop=True,
                             skip_group_check=True)
            rb = ntemps.tile([128, 512],
                             dt.bfloat16 if rbf16 else dt.float32, tag="rb")
            if rbf16:
                with nc.allow_low_precision(reason="softmax denom bf16"):
                    nc.vector.reciprocal(rb, ps_b)
            else:
                nc.vector.reciprocal(rb, ps_b)
            nc.vector.tensor_tensor(oTs[(h, b)][:, q0:q0 + 512], ps_os[h], rb,
                                    mybir.AluOpType.mult)

    pending = None  # outproj runs one q-block behind SDPA to hide norm latency
    genq = []       # kco mode: queue of in-flight outproj generators
    pend2 = []      # qpair mode: two outproj blocks behind

    def _chain(gens):
        for g in gens:
            yield from g

    def adv(n):
        done = 0
        while done < n and genq:
            try:
                next(genq[0])
                done += 1
            except StopIteration:
                genq.pop(0)

    for b in range(B):
        if pmode in ("pair", "quad"):
            gp = 2 if pmode == "pair" else 4
            for tp in range(4 // gp):
                if vorder == "smart":
                    vp = "mid" if (b == 0 and tp == 0) else "first"
                else:
                    vp = vorder
                proj_pair(b, tp, gp, vpos=vp)
                if b == 0 and tp == 0:
                    nc.sync.dma_start(
                        out=wo_sb, in_=wo_d.rearrange("(h p) e -> p h e", p=128))
                    if mmask:
                        nc.sync.dma_start(out=mskt_sb, in_=mskt_d[:])
                        nc.sync.dma_start(out=id_sb, in_=id_d[:])
                    else:
                        nc.sync.dma_start(
                            out=msk_sb,
                            in_=msk_d.rearrange("m p t -> p m t"))
        else:
            for ttl in range(4):
                proj_tile(b, ttl)
                if b == 0 and ttl == 0:
                    # late consts (not needed until sdpa/outproj)
                    nc.sync.dma_start(
                        out=wo_sb, in_=wo_d.rearrange("(h p) e -> p h e", p=128))
                    if mmask:
                        nc.sync.dma_start(out=mskt_sb, in_=mskt_d[:])
                        nc.sync.dma_start(out=id_sb, in_=id_d[:])
                    else:
                        nc.sync.dma_start(
                            out=msk_sb,
                            in_=msk_d.rearrange("m p t -> p m t"))
        if smode == "qpair":
            # per half-batch: both q-blocks of h=0 first, then h=1 --
            # h=1's K/rope wait at SDPA start is covered by h=0's work
            for qp in range(2):
                qbs = (2 * qp, 2 * qp + 1)
                inters = [outproj_gen(*p) for p in pend2]
                inter = _chain(inters) if inters else None
                if inter is not None and b == 1 and qp == 0:
                    for _ in range(pre0):
                        next(inter, None)
                nk_tot = sum(2 * 4 * (qb + 1) for qb in qbs)
                per = (len(inters) * 16.0 / oq) / nk_tot
                for h in range(HPC):
                    for qb in qbs:
                        sdpa_block(b, qb, h, inter=inter, per_kc=per)
                if inter is not None:
                    for _ in inter:
                        pass
                pend2 = [(b, qbs[0]), (b, qbs[1])]
            continue
        if smode == "dual":
            for qb in range(4):
                inter = outproj_gen(*pending) if pending is not None else None
                sdpa_dual(b, qb, inter, pre=(pre0 if qb == 0 else 0))
                if inter is not None:
                    for _ in inter:
                        pass
                pending = (b, qb)
            continue
        if smode == "kco":
            kco_pass(b, 0, adv, None)
            kco_pass(b, 1, adv,
                     lambda qb, b=b: genq.append(outproj_gen(b, qb)))
            continue
        for qb in range(4):
            inter = outproj_gen(*pending) if pending is not None else None
            nk = 4 * (qb + 1)
            if spread2:
                ny = 16.0 / oq
                sdpa_block(b, qb, 0, inter=inter, per_kc=ny / (2 * nk))
                sdpa_block(b, qb, 1, inter=inter, per_kc=ny / (2 * nk))
            else:
                sdpa_block(b, qb, 0)
                sdpa_block(b, qb, 1, inter=inter, per_kc=16.0 / oq / nk)
            if inter is not None:
                for _ in inter:
                    pass
            pending = (b, qb)
    if smode == "kco":
        adv(10 ** 9)
    elif smode == "qpair":
        for p in pend2:
            for _ in outproj_gen(*p):
                pass
    else:
        for _ in outproj_gen(*pending):
            pass


def build_nc(reps=1, use_pool=True, out_bf16=True, dma_rot=True,
             spread2=True, pb=4, ob=6, sbufs=5, osplit=6, oq=2,
             lmode="mm1", lacc="dve2", vmode="dmat", ab=None, trim=True,
             pmode="quad", xb=None, strip_ldw=True, smode="qbo",
             rbf16=True, p2p=False, p2s=False, p2o=False, sb2=None,
             tb=4, nb=2, po=2, mmask=True, oeng="avv", vorder="smart",
             pre0=4, early_l=False, peng="aa"):
    """Build the Bass module. reps>1 wraps the body in a For_i loop executing
    it that many times (used only for wall-clock timing measurements)."""
    import concourse.bass as bass
    import concourse.mybir as mybir
    import concourse.tile as tile

    dt = mybir.dt
    nc = bass.Bass("TRN2", target_bir_lowering=False, debug=False,
                   num_devices=N_CORES)

    xt_d = nc.dram_tensor("xt", [D, TOK], dt.bfloat16, kind="ExternalInput")
    wq_d = nc.dram_tensor("wq", [D, 256], dt.bfloat16, kind="ExternalInput")
    wk_d = nc.dram_tensor("wk", [D, 256], dt.bfloat16, kind="ExternalInput")
    wv_d = nc.dram_tensor("wv", [D, 256], dt.bfloat16, kind="ExternalInput")
    wo_d = nc.dram_tensor("wo", [256, D], dt.bfloat16, kind="ExternalInput")
    cs_d = nc.dram_tensor("cs", [128, T], dt.bfloat16, kind="ExternalInput")
    sn_d = nc.dram_tensor("sn", [128, T], dt.bfloat16, kind="ExternalInput")
    sns_d = nc.dram_tensor("sns", [128, T], dt.bfloat16, kind="ExternalInput")
    pt_d = nc.dram_tensor("pt", [128, 128], dt.bfloat16, kind="ExternalInput")
    onec_d = nc.dram_tensor("onec", [128, 1], dt.bfloat16, kind="ExternalInput")
    oner_d = nc.dram_tensor("oner", [1, 128], dt.bfloat16, kind="ExternalInput")
    msk_d = nc.dram_tensor("msk", [4, 128, 512], dt.bfloat16, kind="ExternalInput")
    mskt_d = nc.dram_tensor("mskt", [128, 128], dt.bfloat16, kind="ExternalInput")
    id_d = nc.dram_tensor("id", [128, 128], dt.bfloat16, kind="ExternalInput")
    out_d = nc.dram_tensor("out", [D, TOK],
                           dt.bfloat16 if out_bf16 else dt.float32,
                           kind="ExternalOutput")
    io = (xt_d, wq_d, wk_d, wv_d, wo_d, cs_d, sn_d, sns_d, pt_d, onec_d,
          oner_d, msk_d, mskt_d, id_d, out_d)

    with tile.TileContext(nc) as tc:
        import contextlib
        with contextlib.ExitStack() as ctx:
            consts = ctx.enter_context(tc.tile_pool(name="consts", bufs=1))
            xpool = ctx.enter_context(tc.tile_pool(
                name="xpool", bufs=(xb or {"single": 2, "pair": 3,
                                           "quad": 4}[pmode])))
            big = ctx.enter_context(tc.tile_pool(name="big", bufs=1))
            temps = ctx.enter_context(tc.tile_pool(name="temps", bufs=tb))
            ntemps = ctx.enter_context(tc.tile_pool(name="ntemps", bufs=nb))
            probsp = ctx.enter_context(tc.tile_pool(name="probs", bufs=pb))
            outp = ctx.enter_context(tc.tile_pool(name="outp", bufs=ob))
            if ab is None:
                ab = {"kco": 6, "dual": 4}.get(smode, 2)
            accp = (ctx.enter_context(tc.tile_pool(name="accp", bufs=ab))
                    if lmode != "mm" else None)
            n_po = 4 if smode == "kco" else po
            n_pl = 8 - 2 - sbufs if lmode == "mm" else 0
            # bank budget: n_po (psum_o) + n_pl + n_s1 ("s") + 2*n_s2 ("s2") = 8
            uses_s2 = p2p or p2s or p2o
            all_p2 = p2p and p2s and p2o
            if not uses_s2:
                n_s1 = sbufs if lmode == "mm" else 8 - n_po
                n_s2 = 0
            elif all_p2:
                n_s1, n_s2 = 0, (8 - n_po) // 2
            else:
                n_s2 = sb2 or 1
                n_s1 = 8 - n_po - n_pl - 2 * n_s2
            psum = (ctx.enter_context(tc.tile_pool(
                name="psum", bufs=n_s1, space="PSUM")) if n_s1 else None)
            psum2 = (ctx.enter_context(tc.tile_pool(
                name="psum2", bufs=n_s2, space="PSUM")) if n_s2 else None)
            psum_o = ctx.enter_context(tc.tile_pool(name="psum_o", bufs=n_po,
                                                    space="PSUM"))
            psum_l = (ctx.enter_context(tc.tile_pool(name="psum_l", bufs=n_pl,
                                                     space="PSUM"))
                      if n_pl else None)
            pools = (consts, xpool, big, temps, ntemps, probsp, outp, accp,
                     psum, psum2, psum_o, psum_l)
            kw = dict(use_pool=use_pool, out_bf16=out_bf16, dma_rot=dma_rot,
                      spread2=spread2, osplit=osplit, oq=oq,
                      lmode=lmode, lacc=lacc, vmode=vmode, trim=trim,
                      pmode=pmode, smode=smode, rbf16=rbf16,
                      p2p=p2p, p2s=p2s, p2o=p2o, mmask=mmask, oeng=oeng,
                      vorder=vorder, pre0=pre0, early_l=early_l, peng=peng)
            if reps > 1:
                with tc.For_i(0, reps, 1):
                    _emit_body(nc, tc, pools, io, **kw)
            else:
                _emit_body(nc, tc, pools, io, **kw)

    if strip_ldw:
        _strip_redundant_ldw(nc)
    return nc


def _strip_redundant_ldw(nc):
    """Drop InstLdweights that reload the exact weights already resident in
    the PE array (same physical AP as the previous load in the block).
    Waits from dropped loads are merged into the next kept instruction."""
    import concourse.mybir as mybir

    n = 0
    for f in nc.m.functions:
        for bb in f.blocks:
            insts = bb.instructions
            new = []
            last_key = None
            pending_waits = []
            for inst in insts:
                if isinstance(inst, mybir.InstLdweights):
                    ap = inst.ins[0]
                    key = (str(ap.ap), ap.offset, ap.memref,
                           str(ap.dtype), str(inst.perf_mode),
                           str(inst.is_transpose),
                           str(getattr(inst, "tile_position", None)))
                    si = inst.sync_info
                    has_update = si is not None and si.on_update
                    if key == last_key and not has_update:
                        if si is not None and si.on_wait:
                            pending_waits.extend(si.on_wait)
                        n += 1
                        continue
                    last_key = key
                    new.append(inst)
                else:
                    if pending_waits and isinstance(inst, mybir.InstMatmult):
                        si = inst.sync_info
                        if si is None:
                            import bass_rust
                            inst.sync_info = bass_rust.SyncInfo(
                                on_wait=list(pending_waits), on_update=[])
                        else:
                            si.on_wait = list(si.on_wait) + pending_waits
                            inst.sync_info = si
                        pending_waits = []
                    new.append(inst)
            assert not pending_waits, "dropped LDW waits had no following matmul"
            bb.instructions = new
    return n


def _split_sync_waits(nc, max_waits=1):
    """Walrus in this env rejects instructions with too many sync waits.
    Hoist excess waits onto preceding same-engine nops."""
    import bass_rust
    import concourse.mybir as mybir

    n_split = 0
    for f in nc.m.functions:
        for bb in f.blocks:
            insts = bb.instructions
            new = []
            dirty = False
            for inst in insts:
                si = inst.sync_info
                if si is not None and si.on_wait and len(si.on_wait) > max_waits:
                    waits = list(si.on_wait)
                    for j, w in enumerate(waits[:-max_waits]):
                        n = mybir.InstNoOp(name=f"{inst.name}-wsplit{j}",
                                           ins=[], outs=[])
                        n.engine = inst.engine
                        n.sync_info = bass_rust.SyncInfo(on_wait=[w], on_update=[])
                        new.append(n)
                        n_split += 1
                    si.on_wait = waits[-max_waits:]
                    inst.sync_info = si
                    dirty = True
                new.append(inst)
            if dirty:
                bb.instructions = new
    return n_split


def _host_prep(x, cos, sin, Wqkv, Wout):
    """Shard + lay out inputs for each core. Returns in_maps list."""
    xf = np.ascontiguousarray(x.reshape(TOK, D).T).astype(BF16)        # [D, TOK]
    csT = np.ascontiguousarray(cos.T)                                   # [128, T]
    snT = np.ascontiguousarray(sin.T)
    cs4 = csT.astype(BF16)                                              # [128, T]
    sn4 = snT.astype(BF16)
    sns = np.concatenate([-snT[:64], snT[64:]], axis=0).astype(BF16)

    # rotate_half permutation: rot = P @ u ; pt = P.T
    P = np.zeros((128, 128), dtype=np.float32)
    P[np.arange(64), np.arange(64) + 64] = -1.0
    P[np.arange(64) + 64, np.arange(64)] = 1.0
    pt = np.ascontiguousarray(P.T).astype(BF16)

    onec = np.ones((128, 1), dtype=np.float32).astype(BF16)
    oner = np.ones((1, 128), dtype=np.float32).astype(BF16)

    msk = np.zeros((4, 128, 512), dtype=np.float32)
    for m in range(4):
        off = m * 128
        kk = np.arange(128)[:, None]
        qq = np.arange(512)[None, :]
        msk[m] = (off + kk <= qq).astype(np.float32)
    msk = msk.astype(BF16)

    # PE-side causal bias: psum += mskt.T @ id adds -1e5 above the diagonal
    kk = np.arange(128)[:, None]
    qq = np.arange(128)[None, :]
    mbias = np.where(kk <= qq, 0.0, -1e5).astype(np.float32)    # [k, q]
    mskt = np.ascontiguousarray(mbias.T).astype(BF16)
    ident = np.eye(128, dtype=np.float32).astype(BF16)

    in_maps = []
    for c in range(N_CORES):
        r0 = c * HPC * HD
        r1 = (c + 1) * HPC * HD
        wq = np.ascontiguousarray(Wqkv[r0:r1, :].T).astype(BF16)          # [D, 256]
        wk = np.ascontiguousarray(Wqkv[D + r0:D + r1, :].T).astype(BF16)
        wv = np.ascontiguousarray(Wqkv[2 * D + r0:2 * D + r1, :].T).astype(BF16)
        wo = np.ascontiguousarray(Wout[:, r0:r1].T).astype(BF16)          # [256, D]
        in_maps.append({
            "xt": xf, "wq": wq, "wk": wk, "wv": wv, "wo": wo,
            "cs": cs4, "sn": sn4, "sns": sns, "pt": pt, "onec": onec,
            "oner": oner,
            "msk": msk, "mskt": mskt, "id": ident,
        })
    return in_maps


def kernel(x, cos, sin, Wqkv, Wout):
    from concourse.bass_utils import run_bass_kernel_spmd

    x = np.asarray(x, dtype=np.float32)
    cos = np.asarray(cos, dtype=np.float32)
    sin = np.asarray(sin, dtype=np.float32)
    Wqkv = np.asarray(Wqkv, dtype=np.float32)
    Wout = np.asarray(Wout, dtype=np.float32)

    if "nc" not in _CACHE:
        nc = build_nc()
        _split_sync_waits(nc, max_waits=1)
        _CACHE["nc"] = nc
    nc = _CACHE["nc"]

    in_maps = _host_prep(x, cos, sin, Wqkv, Wout)
    res = run_bass_kernel_spmd(nc, in_maps, core_ids=list(range(N_CORES)))
    acc = np.zeros((D, TOK), dtype=np.float32)
    for c in range(N_CORES):
        acc += np.asarray(res.results[c]["out"], dtype=np.float32)
    return np.ascontiguousarray(acc.T).reshape(B, T, D)



# revision 52
# speedup vs baseline: 1.0129x; 1.0129x over previous
"""Trainium2 Bass kernel for causal multi-head attention with RoPE.

Problem shapes (hardcoded): x [2,2048,2048] f32, Wqkv [6144,2048], Wout [2048,2048],
cos/sin [2048,128]. 16 heads x 128 head-dim.

Sharding: tensor-parallel over heads -- 2 heads per core on 8 cores.
Each core computes qkv projection for its heads, RoPE, causal SDPA, and its
slice of the output projection (row-parallel); host sums the 8 partials.

All on-device layouts keep tokens on the free dimension ([dim, tokens]) so no
transposes are ever needed:
  - Q/K produced as qT/kT [hd, tok] directly from the projection.
  - V produced as v [tok, hd] (other matmul orientation).
  - scoresT [k_tok, q_tok] = kT_tile.T @ qT -> softmax over the partition dim:
    no max subtraction (scores are provably bounded ~N(0,1)), l = column sums
    via a ones-vector matmul, normalization applied to the attention output.
  - attention output oT [hd, q_tok] = v_chunk.T @ probsT, accumulated in PSUM.
  - output projection outT [D, tok] = WoutT_chunk.T @ oT.
RoPE rotate_half is a fixed +-1 permutation => done with a 128x128 matmul.
"""

import numpy as np
import ml_dtypes

B, T, D, H = 2, 2048, 2048, 16
HD = 128
N_CORES = 8
HPC = H // N_CORES          # heads per core = 2
TOK = B * T                 # 4096 flattened tokens
NT = TOK // 512             # 8 token tiles of 512
KC = D // 128               # 16 contraction chunks for the projections
SCALE = 1.0 / float(np.sqrt(HD))

BF16 = ml_dtypes.bfloat16

_CACHE = {}


def _emit_body(nc, tc, pools, io, use_pool=False, out_bf16=True,
               dma_rot=True, spread2=False, osplit=2, oq=2,
               lmode="mm", lacc="dve", vmode="x", trim=True,
               pmode="single", smode="qbo", rbf16=False,
               p2p=False, p2s=False, p2o=False,
               mmask=False, oeng=None, vorder="last", pre0=4,
               early_l=False, peng="aa", odma2=False):
    """Emit one full forward pass, batch-pipelined."""
    import concourse.bass as bass  # noqa: F401
    import concourse.mybir as mybir

    dt = mybir.dt
    (consts, xpool, big, temps, ntemps, probsp, outp, accp, psum, psum2,
     psum_o, psum_l) = pools
    (xt_d, wq_d, wk_d, wv_d, wo_d, cs_d, sn_d, sns_d, pt_d, onec_d, oner_d,
     msk_d, mskt_d, id_d, out_d) = io

    # ---- resident constants / weights in SBUF ----
    # startup order matters: the first proj matmul needs (wqa chunk 0, xt
    # chunk 0) — emit those DMAs first in small pieces so it can issue early;
    # cos/sin aren't read until the first rope epilogue ~20us later.
    wq_r = wq_d.rearrange("(c p) e -> p c e", p=128)
    wqa_sb = consts.tile([128, KC // 2, 256], dt.bfloat16, tag="wqa")
    wqb_sb = consts.tile([128, KC // 2, 256], dt.bfloat16, tag="wqb")
    wk_sb = consts.tile([128, KC, 256], dt.bfloat16, tag="wk")
    wv_sb = consts.tile([128, KC, 256], dt.bfloat16, tag="wv")
    nc.sync.dma_start(out=wqa_sb[:, 0:2, :], in_=wq_r[:, 0:2, :])
    cs_sb = consts.tile([128, T], dt.bfloat16, tag="cs")
    sn_sb = consts.tile([128, T], dt.bfloat16, tag="sn")
    if not dma_rot:
        pt_sb = consts.tile([128, 128], dt.bfloat16, tag="pt")
        nc.sync.dma_start(out=pt_sb, in_=pt_d[:])
    if lmode == "mm":
        onec_sb = consts.tile([128, 1], dt.bfloat16, tag="onec")
        nc.sync.dma_start(out=onec_sb, in_=onec_d[:])
        oner_sb = consts.tile([1, 128], dt.bfloat16, tag="oner")
        nc.sync.dma_start(out=oner_sb, in_=oner_d[:])
    if lmode == "mm1":
        # all-ones [128,128]: lhsT for the column-sum+broadcast matmul
        ones_sb = consts.tile([128, 128], dt.bfloat16, tag="ones")
        nc.vector.memset(ones_sb, 1.0)
    wo_sb = consts.tile([128, HPC, D], dt.bfloat16, tag="wo")
    msk_sb = (consts.tile([128, 4, 512], dt.bfloat16, tag="msk", name="msk")
              if not mmask else None)
    if mmask:
        # causal mask as PE psum-accumulate: mskt = (step mask).T, id = I128
        mskt_sb = consts.tile([128, 128], dt.bfloat16, tag="mskt")
        id_sb = consts.tile([128, 128], dt.bfloat16, tag="id")

    # ---- resident activations: per-(head,batch) for fine-grained deps ----
    qTs = {(h, b): big.tile([128, T], dt.bfloat16, tag=f"qT{h}{b}", name=f"qT{h}{b}")
           for h in range(HPC) for b in range(B)}
    kTs = {(h, b): big.tile([128, T], dt.bfloat16, tag=f"kT{h}{b}", name=f"kT{h}{b}")
           for h in range(HPC) for b in range(B)}
    oTs = {(h, b): big.tile([128, T], dt.bfloat16, tag=f"oT{h}{b}", name=f"oT{h}{b}")
           for h in range(HPC) for b in range(B)}
    vss = {b: big.tile([128, 16, 256], dt.bfloat16, tag=f"v{b}", name=f"v{b}")
           for b in range(B)}

    xt_r = xt_d.rearrange("(c p) t -> p c t", p=128)  # [128, 16, 4096]
    xt_first = []
    for xh in range(2):
        xt_sb = xpool.tile([128, KC // 2, 512], dt.bfloat16, tag=f"xt{xh}",
                           name=f"xtp{xh}")
        if xh == 0:
            # chunk 0 on the Act HWDGE queue: lands in parallel with the
            # weight chunk on the SP queue, so matmul 0 starts earliest
            nc.scalar.dma_start(out=xt_sb[:, 0:2, :], in_=xt_r[:, 0:2, 0:512])
            nc.sync.dma_start(out=xt_sb[:, 2:, :], in_=xt_r[:, 2:8, 0:512])
        else:
            nc.sync.dma_start(out=xt_sb, in_=xt_r[:, xh * 8:(xh + 1) * 8, 0:512])
        xt_first.append(xt_sb)
    nc.sync.dma_start(out=wqa_sb[:, 2:, :], in_=wq_r[:, 2:KC // 2, :])

    def late_consts():
        nc.sync.dma_start(out=wqb_sb, in_=wq_r[:, KC // 2:, :])
        nc.sync.dma_start(out=cs_sb, in_=cs_d[:])
        nc.sync.dma_start(out=sn_sb, in_=(sns_d[:] if dma_rot else sn_d[:]))
        if vorder == "smart":
            # V projection runs before K: load wv ahead of wk
            nc.sync.dma_start(out=wv_sb,
                              in_=wv_d.rearrange("(c p) e -> p c e", p=128))
            nc.sync.dma_start(out=wk_sb,
                              in_=wk_d.rearrange("(c p) e -> p c e", p=128))
        else:
            nc.sync.dma_start(out=wk_sb,
                              in_=wk_d.rearrange("(c p) e -> p c e", p=128))
            nc.sync.dma_start(out=wv_sb,
                              in_=wv_d.rearrange("(c p) e -> p c e", p=128))

    def _load_xts(t0g, split_first=False):
        xts = []
        for xh in range(2):
            xt_sb = xpool.tile([128, KC // 2, 512], dt.bfloat16,
                               tag=f"xt{xh}", name=f"xt{xh}")
            if xh == 0 and split_first:
                nc.sync.dma_start(out=xt_sb[:, 0:2, :],
                                  in_=xt_r[:, 0:2, t0g:t0g + 512])
                nc.sync.dma_start(out=xt_sb[:, 2:, :],
                                  in_=xt_r[:, 2:8, t0g:t0g + 512])
            else:
                nc.sync.dma_start(
                    out=xt_sb, in_=xt_r[:, xh * 8:(xh + 1) * 8, t0g:t0g + 512])
            xts.append(xt_sb)
        return xts

    def proj_pair(b, tp, gp=2, vpos="last"):
        """Project gp adjacent 512-token tiles; the kc loop issues the tiles'
        matmuls back-to-back so each weight chunk is loaded once. vpos places
        the V projection first/mid/last among the three so its DMA transposes
        into vss finish before SDPA's o-matmuls need them."""
        tls = tuple(tp * gp + j for j in range(gp))
        t0s = [ttl * 512 for ttl in tls]
        if b == 0 and tp == 0:
            xtss = [xt_first] + [_load_xts(512 * ttl, split_first=True)
                                 for ttl in tls[1:]]
            late_consts()
        else:
            xtss = [_load_xts((b * 4 + ttl) * 512) for ttl in tls]

        def epilogue(ps, dsts, m, t0, raw_in=False, ceng="a"):
            if raw_in:
                raw = ps
            else:
                raw = temps.tile([128, 512], dt.bfloat16, tag="raw")
                if ceng == "v":
                    nc.vector.tensor_copy(out=raw, in_=ps)
                else:
                    nc.scalar.copy(out=raw, in_=ps)
            rsb = temps.tile([128, 512], dt.bfloat16, tag="rsb")
            nc.sync.dma_start(out=rsb[0:64, :], in_=raw[64:128, :])
            nc.sync.dma_start(out=rsb[64:128, :], in_=raw[0:64, :])
            t1 = temps.tile([128, 512], dt.bfloat16, tag="t1")
            nc.vector.tensor_tensor(t1, raw, cs_sb[:, t0:t0 + 512],
                                    mybir.AluOpType.mult)
            t2 = temps.tile([128, 512], dt.bfloat16, tag="t2")
            nc.vector.tensor_tensor(t2, rsb, sn_sb[:, t0:t0 + 512],
                                    mybir.AluOpType.mult)
            eng = nc.gpsimd if use_pool else nc.vector
            eng.tensor_tensor(dsts[(m, b)][:, t0:t0 + 512], t1, t2,
                              mybir.AluOpType.add)

        def alloc_ps(nm):
            if p2p:
                t2 = psum2.tile([128, 2, 512], dt.float32, tag="s2", name=nm)
                return [t2[:, i, :] for i in range(2)], t2
            return [psum.tile([128, 512], dt.float32, tag="s", name=f"{nm}{i}")
                    for i in range(gp)], None

        def qk_phase(w_sb, dsts, ceng="a"):
            for m in range(HPC):
                pss, ps2 = alloc_ps("psp")
                for kc in range(KC):
                    if isinstance(w_sb, tuple):
                        w_ap = (w_sb[1] if kc < 8 else w_sb[2])[:, kc % 8,
                                                               m * 128:(m + 1) * 128]
                    else:
                        w_ap = w_sb[:, kc, m * 128:(m + 1) * 128]
                    for i in range(gp):
                        nc.tensor.matmul(
                            pss[i], lhsT=w_ap,
                            rhs=xtss[i][kc // 8][:, kc % 8, :],
                            start=(kc == 0), stop=(kc == KC - 1),
                            skip_group_check=(i > 0 and p2p),
                        )
                if p2p:
                    raw2 = temps.tile([128, 2, 512], dt.bfloat16, tag="raw2")
                    nc.scalar.copy(out=raw2, in_=ps2)
                    for i in range(gp):
                        epilogue(raw2[:, i, :], dsts, m, t0s[i], raw_in=True)
                else:
                    for i in range(gp):
                        epilogue(pss[i], dsts, m, t0s[i], ceng=ceng)

        def v_phase():
            for m in range(HPC):
                pss, ps2 = alloc_ps("psv")
                for kc in range(KC):
                    w_ap = wv_sb[:, kc, m * 128:(m + 1) * 128]
                    for i in range(gp):
                        nc.tensor.matmul(
                            pss[i], lhsT=w_ap,
                            rhs=xtss[i][kc // 8][:, kc % 8, :],
                            start=(kc == 0), stop=(kc == KC - 1),
                            skip_group_check=(i > 0 and p2p),
                        )
                if p2p:
                    vtmp2 = temps.tile([128, 2, 512], dt.bfloat16, tag="raw2")
                    nc.scalar.copy(out=vtmp2, in_=ps2)
                    for i in range(gp):
                        for tcc in range(4):
                            nc.sync.dma_start_transpose(
                                out=vss[b][:, tls[i] * 4 + tcc,
                                           m * 128:(m + 1) * 128],
                                in_=vtmp2[:, i, tcc * 128:(tcc + 1) * 128])
                else:
                    for i in range(gp):
                        vtmp = temps.tile([128, 512], dt.bfloat16, tag="raw")
                        nc.scalar.copy(out=vtmp, in_=pss[i])
                        for tcc in range(4):
                            nc.sync.dma_start_transpose(
                                out=vss[b][:, tls[i] * 4 + tcc,
                                           m * 128:(m + 1) * 128],
                                in_=vtmp[:, tcc * 128:(tcc + 1) * 128])

        qph = [(("wqsplit", wqa_sb, wqb_sb), qTs, peng[0]),
               (wk_sb, kTs, peng[1])]
        if vpos == "first":
            v_phase()
        for pi, (w_sb, dsts, ce) in enumerate(qph):
            qk_phase(w_sb, dsts, ceng=ce)
            if pi == 0 and vpos == "mid":
                v_phase()
        if vpos == "last":
            v_phase()

    def proj_tile(b, ttl):
        t0g = (b * 4 + ttl) * 512   # global token offset
        t0 = ttl * 512              # within-batch offset
        if b == 0 and ttl == 0:
            xts = xt_first
            late_consts()
        else:
            xts = _load_xts(t0g)

        for w_sb, dsts in ((("wqsplit", wqa_sb, wqb_sb), qTs), (wk_sb, kTs)):
            for m in range(HPC):
                ps = psum.tile([128, 512], dt.float32, tag="s")
                for kc in range(KC):
                    if isinstance(w_sb, tuple):
                        w_ap = (w_sb[1] if kc < 8 else w_sb[2])[:, kc % 8,
                                                               m * 128:(m + 1) * 128]
                    else:
                        w_ap = w_sb[:, kc, m * 128:(m + 1) * 128]
                    nc.tensor.matmul(
                        ps,
                        lhsT=w_ap,
                        rhs=xts[kc // 8][:, kc % 8, :],
                        start=(kc == 0), stop=(kc == KC - 1),
                    )
                raw = temps.tile([128, 512], dt.bfloat16, tag="raw")
                nc.scalar.copy(out=raw, in_=ps)
                if dma_rot:
                    rsb = temps.tile([128, 512], dt.bfloat16, tag="rsb")
                    nc.sync.dma_start(out=rsb[0:64, :], in_=raw[64:128, :])
                    nc.sync.dma_start(out=rsb[64:128, :], in_=raw[0:64, :])
                else:
                    psr = psum.tile([128, 512], dt.float32, tag="s")
                    nc.tensor.matmul(psr, lhsT=pt_sb, rhs=raw, start=True,
                                     stop=True)
                    rsb = temps.tile([128, 512], dt.bfloat16, tag="rsb")
                    nc.scalar.copy(out=rsb, in_=psr)
                t1 = temps.tile([128, 512], dt.bfloat16, tag="t1")
                nc.vector.tensor_tensor(t1, raw, cs_sb[:, t0:t0 + 512],
                                        mybir.AluOpType.mult)
                t2 = temps.tile([128, 512], dt.bfloat16, tag="t2")
                nc.vector.tensor_tensor(t2, rsb, sn_sb[:, t0:t0 + 512],
                                        mybir.AluOpType.mult)
                eng = nc.gpsimd if use_pool else nc.vector
                eng.tensor_tensor(dsts[(m, b)][:, t0:t0 + 512], t1, t2,
                                  mybir.AluOpType.add)

        if vmode == "dmat":
            # V as vT [hd, tok] (512-free matmuls), then DMA-transpose into
            # the [tok, hd] layout the o-matmul needs.
            for m in range(HPC):
                psv = psum.tile([128, 512], dt.float32, tag="s")
                for kc in range(KC):
                    nc.tensor.matmul(
                        psv,
                        lhsT=wv_sb[:, kc, m * 128:(m + 1) * 128],
                        rhs=xts[kc // 8][:, kc % 8, :],
                        start=(kc == 0), stop=(kc == KC - 1),
                    )
                vtmp = temps.tile([128, 512], dt.bfloat16, tag="raw")
                nc.scalar.copy(out=vtmp, in_=psv)
                for tcc in range(4):
                    nc.sync.dma_start_transpose(
                        out=vss[b][:, ttl * 4 + tcc, m * 128:(m + 1) * 128],
                        in_=vtmp[:, tcc * 128:(tcc + 1) * 128])
        else:
            # V -> [tok, hd]; two 256-wide groups share one psum bank
            for pair in range(2):
                psv = psum.tile([128, 512], dt.float32, tag="s")
                for half in range(2):
                    sub = pair * 2 + half
                    for kc in range(KC):
                        nc.tensor.matmul(
                            psv[:, half * 256:(half + 1) * 256],
                            lhsT=xts[kc // 8][:, kc % 8, sub * 128:(sub + 1) * 128],
                            rhs=wv_sb[:, kc, :],
                            start=(kc == 0 and half == 0),
                            stop=(kc == KC - 1),
                            skip_group_check=(half == 1),
                        )
                nc.scalar.copy(
                    out=vss[b][:, ttl * 4 + pair * 2: ttl * 4 + pair * 2 + 2, :],
                    in_=psv)

    def kco_pass(b, h, adv, on_done=None):
        """SDPA for all 4 q-blocks of (b,h), k-chunk-outer: the 4 scores
        matmuls share one kT weight load, the 4 o-matmuls share one V load.
        adv(n) advances the woven outproj generator queue."""
        two = lacc in ("dve2", "pd2")
        ps_os = {qb: psum_o.tile([128, 512], dt.float32, tag="acc_o",
                                 name=f"pso{qb}") for qb in range(4)}
        if two:
            accs = {qb: accp.tile([128, 2, 512], dt.bfloat16, tag="acc",
                                  name=f"acc{qb}") for qb in range(4)}
            aengs = ((nc.vector, nc.vector) if lacc == "dve2"
                     else (nc.gpsimd, nc.vector))
        else:
            accs = {qb: accp.tile([128, 512], dt.bfloat16, tag="acc",
                                  name=f"acc{qb}") for qb in range(4)}
            aeng = nc.gpsimd if lacc == "pool" else nc.vector
        for kc in range(16):
            qlo = kc // 4
            prs = {}
            for qb in range(qlo, 4):
                moff = kc - 4 * qb
                qs = max(moff, 0) * 128 if trim else 0
                q0 = qb * 512
                diag = mmask and moff >= 0
                ps_s = psum.tile([128, 512], dt.float32, tag="s",
                                 name="ps_s")
                nc.tensor.matmul(
                    ps_s[:, qs:],
                    lhsT=kTs[(h, b)][:, kc * 128:(kc + 1) * 128],
                    rhs=qTs[(h, b)][:, q0 + qs:q0 + 512],
                    start=True, stop=not diag,
                )
                if diag:
                    ms = moff * 128
                    nc.tensor.matmul(
                        ps_s[:, ms:ms + 128], lhsT=mskt_sb, rhs=id_sb,
                        start=False, stop=True, skip_group_check=True,
                    )
                pr = probsp.tile([128, 512], dt.bfloat16, tag="probs")
                nc.scalar.activation(pr[:, qs:], ps_s[:, qs:],
                                     mybir.ActivationFunctionType.Exp,
                                     scale=SCALE)
                if moff >= 0 and not mmask:
                    eng = nc.gpsimd if use_pool else nc.vector
                    eng.tensor_tensor(pr[:, qs:qs + 128], pr[:, qs:qs + 128],
                                      msk_sb[:, moff, qs:qs + 128],
                                      mybir.AluOpType.mult)
                if two:
                    c = kc % 2
                    if kc < 2:
                        aengs[c].tensor_copy(out=accs[qb][:, c, qs:],
                                             in_=pr[:, qs:])
                    else:
                        aengs[c].tensor_tensor(accs[qb][:, c, qs:],
                                               accs[qb][:, c, qs:],
                                               pr[:, qs:],
                                               mybir.AluOpType.add)
                elif kc == 0:
                    aeng.tensor_copy(out=accs[qb], in_=pr)
                else:
                    aeng.tensor_tensor(accs[qb][:, qs:], accs[qb][:, qs:],
                                       pr[:, qs:], mybir.AluOpType.add)
                prs[qb] = (pr, qs)
            for qb in range(qlo, 4):
                pr, qs = prs[qb]
                nc.tensor.matmul(
                    ps_os[qb][:, qs:],
                    lhsT=vss[b][:, kc, h * 128:(h + 1) * 128],
                    rhs=pr[:, qs:],
                    start=(kc == 0), stop=(kc == 4 * qb + 3),
                    skip_group_check=(qs > 0),
                )
            if kc % 4 == 3:
                # q-block qlo is complete: normalize it
                qb = qlo
                ps_b = psum.tile([128, 512], dt.float32, tag="s",
                                 name="ps_b")
                if two:
                    qc1 = 128 if qb == 0 else 0
                    nc.tensor.matmul(ps_b, lhsT=ones_sb, rhs=accs[qb][:, 0, :],
                                     start=True, stop=False)
                    nc.tensor.matmul(ps_b[:, qc1:], lhsT=ones_sb,
                                     rhs=accs[qb][:, 1, qc1:],
                                     start=False, stop=True,
                                     skip_group_check=True)
                else:
                    nc.tensor.matmul(ps_b, lhsT=ones_sb, rhs=accs[qb],
                                     start=True, stop=True)
                rb = ntemps.tile([128, 512], dt.float32, tag="rb")
                nc.vector.reciprocal(rb, ps_b)
                nc.vector.tensor_tensor(oTs[(h, b)][:, qb * 512:(qb + 1) * 512],
                                        ps_os[qb], rb, mybir.AluOpType.mult)
                if on_done is not None:
                    on_done(qb)
            adv(2 if h == 1 else 1)

    def sdpa_block(b, qb, h, inter=None, per_kc=0.0):
        q0 = qb * 512
        nk = 4 * (qb + 1)
        steps = 0.0
        two = lacc in ("dve2", "pd2", "pp2")
        ps_b_early = None
        ps_o = psum_o.tile([128, 512], dt.float32, tag="acc_o")
        if lmode == "mm":
            ps_l = psum_l.tile([1, 512], dt.float32, tag="acc_l")
        elif two:
            acc = accp.tile([128, 2, 512], dt.bfloat16, tag="acc")
            aengs = {"dve2": (nc.vector, nc.vector),
                     "pd2": (nc.gpsimd, nc.vector),
                     "pp2": (nc.gpsimd, nc.gpsimd)}[lacc]
            first_qs = {}
        else:
            acc = accp.tile([128, 512], dt.bfloat16, tag="acc")
            if lacc == "mixed":
                aeng = nc.vector if qb >= 2 else nc.gpsimd
            else:
                aeng = nc.gpsimd if lacc == "pool" else nc.vector
        def qs_of(kc):
            moff = kc - 4 * qb
            return max(moff, 0) * 128 if trim else 0, moff

        def post_exp(kc, j, pr_ap):
            """mask + l-accumulate + o-matmul for one kc chunk."""
            nonlocal steps, ps_b_early
            qs, moff = qs_of(kc)
            if moff >= 0 and not mmask:
                eng = nc.gpsimd if use_pool else nc.vector
                eng.tensor_tensor(pr_ap[:, qs:qs + 128], pr_ap[:, qs:qs + 128],
                                  msk_sb[:, moff, qs:qs + 128],
                                  mybir.AluOpType.mult)
            if two:
                c = kc % 2
                if kc < 2:
                    first_qs[c] = qs
                    aengs[c].tensor_copy(out=acc[:, c, qs:], in_=pr_ap[:, qs:])
                else:
                    aengs[c].tensor_tensor(acc[:, c, qs:], acc[:, c, qs:],
                                           pr_ap[:, qs:], mybir.AluOpType.add)
            elif kc == 0:
                aeng.tensor_copy(out=acc, in_=pr_ap)
            else:
                aeng.tensor_tensor(acc[:, qs:], acc[:, qs:], pr_ap[:, qs:],
                                   mybir.AluOpType.add)
            if two and early_l and lmode != "mm":
                # chain c's last add lands at kc = nk-2+c: issue its l-sum
                # matmul immediately so the block-end ps_b wait shrinks
                if kc == nk - 2:
                    ps_b_early = psum.tile([128, 512], dt.float32, tag="s",
                                           name="ps_be")
                    nc.tensor.matmul(ps_b_early, lhsT=ones_sb,
                                     rhs=acc[:, 0, :], start=True, stop=False)
                elif kc == nk - 1:
                    qc1 = first_qs[1]
                    nc.tensor.matmul(ps_b_early[:, qc1:], lhsT=ones_sb,
                                     rhs=acc[:, 1, qc1:], start=False,
                                     stop=True, skip_group_check=True)
            nc.tensor.matmul(
                ps_o[:, qs:],
                lhsT=vss[b][:, kc, h * 128:(h + 1) * 128],
                rhs=pr_ap[:, qs:],
                start=(kc == 0), stop=(kc == nk - 1),
                skip_group_check=(qs > 0),
            )
            if inter is not None:
                steps += per_kc
                while steps >= 1.0:
                    next(inter, None)
                    steps -= 1.0

        def scores_mm(ps_ap, kc, qs, moff, skip_gc=False):
            """scores matmul (+ causal bias via PE accumulate when mmask)."""
            diag = mmask and moff >= 0
            nc.tensor.matmul(
                ps_ap[:, qs:],
                lhsT=kTs[(h, b)][:, kc * 128:(kc + 1) * 128],
                rhs=qTs[(h, b)][:, q0 + qs:q0 + 512],
                start=True, stop=not diag,
                skip_group_check=skip_gc,
            )
            if diag:
                ms = moff * 128
                nc.tensor.matmul(
                    ps_ap[:, ms:ms + 128], lhsT=mskt_sb, rhs=id_sb,
                    start=False, stop=True, skip_group_check=True,
                )

        if p2s:
            for kp in range(nk // 2):
                kcs = (2 * kp, 2 * kp + 1)
                ps2 = psum2.tile([128, 2, 512], dt.float32, tag="s2",
                                 name="ps_s2")
                qss = []
                for j, kc in enumerate(kcs):
                    qs, moff = qs_of(kc)
                    qss.append(qs)
                    scores_mm(ps2[:, j, :], kc, qs, moff, skip_gc=(j > 0))
                qsp = min(qss)
                pr2 = probsp.tile([128, 2, 512], dt.bfloat16, tag="probs")
                nc.scalar.activation(pr2[:, :, qsp:], ps2[:, :, qsp:],
                                     mybir.ActivationFunctionType.Exp,
                                     scale=SCALE)
                for j, kc in enumerate(kcs):
                    post_exp(kc, j, pr2[:, j, :])
        else:
          for kc in range(nk):
            qs, moff = qs_of(kc)
            ps_s = psum.tile([128, 512], dt.float32, tag="s")
            scores_mm(ps_s, kc, qs, moff)
            pr = probsp.tile([128, 512], dt.bfloat16, tag="probs")
            nc.scalar.activation(pr[:, qs:], ps_s[:, qs:],
                                 mybir.ActivationFunctionType.Exp,
                                 scale=SCALE)
            if lmode == "mm":
                if moff >= 0 and not mmask:
                    eng = nc.gpsimd if use_pool else nc.vector
                    eng.tensor_tensor(pr[:, qs:qs + 128], pr[:, qs:qs + 128],
                                      msk_sb[:, moff, qs:qs + 128],
                                      mybir.AluOpType.mult)
                nc.tensor.matmul(ps_l[:, qs:], lhsT=onec_sb, rhs=pr[:, qs:],
                                 start=(kc == 0), stop=(kc == nk - 1),
                                 skip_group_check=(qs > 0))
                nc.tensor.matmul(
                    ps_o[:, qs:],
                    lhsT=vss[b][:, kc, h * 128:(h + 1) * 128],
                    rhs=pr[:, qs:],
                    start=(kc == 0), stop=(kc == nk - 1),
                    skip_group_check=(qs > 0),
                )
                if inter is not None:
                    steps += per_kc
                    while steps >= 1.0:
                        next(inter, None)
                        steps -= 1.0
            else:
                post_exp(kc, 0, pr)
        # normalize: oT = ps_o * recip(broadcast(l))
        if lmode == "mm":
            l_row = ntemps.tile([1, 512], dt.bfloat16, tag="l_row")  # bf16 keeps bcast mm fast
            nc.scalar.copy(out=l_row, in_=ps_l)
            ps_b = psum.tile([128, 512], dt.float32, tag="s")
            nc.tensor.matmul(ps_b, lhsT=oner_sb, rhs=l_row, start=True,
                             stop=True)
        else:
            if two and early_l and ps_b_early is not None:
                ps_b = ps_b_early
            else:
                if p2p and p2s and p2o:
                    ps_b2 = psum2.tile([128, 2, 512], dt.float32, tag="s2",
                                       name="ps_b2")
                    ps_b = ps_b2[:, 0, :]
                else:
                    ps_b = psum.tile([128, 512], dt.float32, tag="s")
                if two:
                    for c in (0, 1):
                        qc = first_qs[c]
                        nc.tensor.matmul(ps_b[:, qc:], lhsT=ones_sb,
                                         rhs=acc[:, c, qc:],
                                         start=(c == 0), stop=(c == 1),
                                         skip_group_check=(c == 1))
                else:
                    nc.tensor.matmul(ps_b, lhsT=ones_sb, rhs=acc, start=True,
                                     stop=True)
        rb = ntemps.tile([128, 512],
                         dt.bfloat16 if rbf16 else dt.float32, tag="rb")
        if rbf16:
            with nc.allow_low_precision(reason="softmax denom bf16"):
                nc.vector.reciprocal(rb, ps_b)
        else:
            nc.vector.reciprocal(rb, ps_b)
        nc.vector.tensor_tensor(oTs[(h, b)][:, q0:q0 + 512], ps_o, rb,
                                mybir.AluOpType.mult)

    out_r = out_d.rearrange("(g p) t -> p g t", p=128)

    def ocopy(dtile, o_ap, ps_ap):
        if oeng is not None:
            ch = oeng[dtile % len(oeng)]
            if ch == "a":
                nc.scalar.copy(out=o_ap, in_=ps_ap)
            elif ch == "v":
                nc.vector.tensor_copy(out=o_ap, in_=ps_ap)
            else:
                nc.gpsimd.tensor_copy(out=o_ap, in_=ps_ap)
        elif dtile % osplit == 0:
            nc.scalar.copy(out=o_ap, in_=ps_ap)
        else:
            nc.vector.tensor_copy(out=o_ap, in_=ps_ap)

    def outproj_gen(b, qb):
        q0 = qb * 512
        t0g = b * T + q0
        if p2o:
            for dp in range(D // 256):   # pairs of 128-wide d tiles
                o_sb = outp.tile([128, 2, 512],
                                 dt.bfloat16 if out_bf16 else dt.float32,
                                 tag="o_sb", name="o_sb")
                ps2 = psum2.tile([128, 2, 512], dt.float32, tag="s2",
                                 name="ps_op2")
                for half in range(2):
                    dtile = dp * 2 + half
                    for h in range(HPC):
                        nc.tensor.matmul(
                            ps2[:, half, :],
                            lhsT=wo_sb[:, h, dtile * 128:(dtile + 1) * 128],
                            rhs=oTs[(h, b)][:, q0:q0 + 512],
                            start=(h == 0), stop=(h == HPC - 1),
                            skip_group_check=(half == 1),
                        )
                ocopy(dp, o_sb, ps2)
                nc.sync.dma_start(
                    out=out_r[:, dp * 2:(dp + 1) * 2, t0g:t0g + 512], in_=o_sb)
                yield
            return
        for dp in range(D // (128 * oq)):   # groups of oq 128-wide d tiles
            o_sb = outp.tile([128, oq, 512],
                             dt.bfloat16 if out_bf16 else dt.float32,
                             tag="o_sb", name="o_sb")
            for half in range(oq):
                dtile = dp * oq + half
                ps = psum.tile([128, 512], dt.float32, tag="s", name="ps_op")
                for h in range(HPC):
                    nc.tensor.matmul(
                        ps,
                        lhsT=wo_sb[:, h, dtile * 128:(dtile + 1) * 128],
                        rhs=oTs[(h, b)][:, q0:q0 + 512],
                        start=(h == 0), stop=(h == HPC - 1),
                    )
                ocopy(dtile, o_sb[:, half, :], ps)
            oeng_dma = nc.scalar if (odma2 and dp % 2) else nc.sync
            oeng_dma.dma_start(
                out=out_r[:, dp * oq:(dp + 1) * oq, t0g:t0g + 512], in_=o_sb)
            yield

    def sdpa_dual(b, qb, inter=None, pre=0):
        """Both heads' SDPA interleaved kc-step by kc-step for one q-block.
        Outproj steps are pre-drained (covering the wait on K/V readiness at
        block 0) then woven between kc steps."""
        q0 = qb * 512
        nk = 4 * (qb + 1)
        if inter is not None:
            for _ in range(pre):
                next(inter, None)
        steps = 0.0
        per_kc = max(16.0 / oq - pre, 0) / nk
        ps_os = {h: psum_o.tile([128, 512], dt.float32, tag="acc_o",
                                name=f"pso{h}") for h in (0, 1)}
        accs = {h: accp.tile([128, 2, 512], dt.bfloat16, tag="acc",
                             name=f"accd{h}") for h in (0, 1)}
        aengs = ((nc.gpsimd, nc.vector) if lacc == "pd2"
                 else (nc.vector, nc.vector))
        for kc in range(nk):
            moff = kc - 4 * qb
            qs = max(moff, 0) * 128 if trim else 0
            diag = mmask and moff >= 0
            prs = {}
            for h in (0, 1):
                ps_s = psum.tile([128, 512], dt.float32, tag="s",
                                 name=f"ps_s{h}")
                nc.tensor.matmul(
                    ps_s[:, qs:],
                    lhsT=kTs[(h, b)][:, kc * 128:(kc + 1) * 128],
                    rhs=qTs[(h, b)][:, q0 + qs:q0 + 512],
                    start=True, stop=not diag,
                )
                if diag:
                    ms = moff * 128
                    nc.tensor.matmul(ps_s[:, ms:ms + 128], lhsT=mskt_sb,
                                     rhs=id_sb, start=False, stop=True,
                                     skip_group_check=True)
                pr = probsp.tile([128, 512], dt.bfloat16, tag="probs",
                                 name=f"pr{h}")
                nc.scalar.activation(pr[:, qs:], ps_s[:, qs:],
                                     mybir.ActivationFunctionType.Exp,
                                     scale=SCALE)
                if not mmask and moff >= 0:
                    eng = nc.gpsimd if use_pool else nc.vector
                    eng.tensor_tensor(pr[:, qs:qs + 128], pr[:, qs:qs + 128],
                                      msk_sb[:, moff, qs:qs + 128],
                                      mybir.AluOpType.mult)
                c = kc % 2
                if kc < 2:
                    aengs[c].tensor_copy(out=accs[h][:, c, qs:],
                                         in_=pr[:, qs:])
                else:
                    aengs[c].tensor_tensor(accs[h][:, c, qs:],
                                           accs[h][:, c, qs:],
                                           pr[:, qs:], mybir.AluOpType.add)
                prs[h] = pr
            for h in (0, 1):
                nc.tensor.matmul(
                    ps_os[h][:, qs:],
                    lhsT=vss[b][:, kc, h * 128:(h + 1) * 128],
                    rhs=prs[h][:, qs:],
                    start=(kc == 0), stop=(kc == nk - 1),
                    skip_group_check=(qs > 0),
                )
            if inter is not None:
                steps += per_kc
                while steps >= 1.0:
                    next(inter, None)
                    steps -= 1.0
        for h in (0, 1):
            ps_b = psum.tile([128, 512], dt.float32, tag="s", name=f"ps_b{h}")
            qc1 = 128 if qb == 0 else 0
            nc.tensor.matmul(ps_b, lhsT=ones_sb, rhs=accs[h][:, 0, :],
                             start=True, stop=False)
            nc.tensor.matmul(ps_b[:, qc1:], lhsT=ones_sb,
                             rhs=accs[h][:, 1, qc1:], start=False, stop=True,
                             skip_group_check=True)
            rb = ntemps.tile([128, 512],
                             dt.bfloat16 if rbf16 else dt.float32, tag="rb")
            if rbf16:
                with nc.allow_low_precision(reason="softmax denom bf16"):
                    nc.vector.reciprocal(rb, ps_b)
            else:
                nc.vector.reciprocal(rb, ps_b)
            nc.vector.tensor_tensor(oTs[(h, b)][:, q0:q0 + 512], ps_os[h], rb,
                                    mybir.AluOpType.mult)

    pending = None  # outproj runs one q-block behind SDPA to hide norm latency
    genq = []       # kco mode: queue of in-flight outproj generators
    pend2 = []      # qpair mode: two outproj blocks behind

    def _chain(gens):
        for g in gens:
            yield from g

    def adv(n):
        done = 0
        while done < n and genq:
            try:
                next(genq[0])
                done += 1
            except StopIteration:
                genq.pop(0)

    for b in range(B):
        if pmode in ("pair", "quad"):
            gp = 2 if pmode == "pair" else 4
            for tp in range(4 // gp):
                if vorder == "smart":
                    vp = "mid" if (b == 0 and tp == 0) else "first"
                else:
                    vp = vorder
                proj_pair(b, tp, gp, vpos=vp)
                if b == 0 and tp == 0:
                    nc.sync.dma_start(
                        out=wo_sb, in_=wo_d.rearrange("(h p) e -> p h e", p=128))
                    if mmask:
                        nc.sync.dma_start(out=mskt_sb, in_=mskt_d[:])
                        nc.sync.dma_start(out=id_sb, in_=id_d[:])
                    else:
                        nc.sync.dma_start(
                            out=msk_sb,
                            in_=msk_d.rearrange("m p t -> p m t"))
        else:
            for ttl in range(4):
                proj_tile(b, ttl)
                if b == 0 and ttl == 0:
                    # late consts (not needed until sdpa/outproj)
                    nc.sync.dma_start(
                        out=wo_sb, in_=wo_d.rearrange("(h p) e -> p h e", p=128))
                    if mmask:
                        nc.sync.dma_start(out=mskt_sb, in_=mskt_d[:])
                        nc.sync.dma_start(out=id_sb, in_=id_d[:])
                    else:
                        nc.sync.dma_start(
                            out=msk_sb,
                            in_=msk_d.rearrange("m p t -> p m t"))
        if smode == "qpair":
            # per half-batch: both q-blocks of h=0 first, then h=1 --
            # h=1's K/rope wait at SDPA start is covered by h=0's work
            for qp in range(2):
                qbs = (2 * qp, 2 * qp + 1)
                inters = [outproj_gen(*p) for p in pend2]
                inter = _chain(inters) if inters else None
                if inter is not None and b == 1 and qp == 0:
                    for _ in range(pre0):
                        next(inter, None)
                nk_tot = sum(2 * 4 * (qb + 1) for qb in qbs)
                per = (len(inters) * 16.0 / oq) / nk_tot
                for h in range(HPC):
                    for qb in qbs:
                        sdpa_block(b, qb, h, inter=inter, per_kc=per)
                if inter is not None:
                    for _ in inter:
                        pass
                pend2 = [(b, qbs[0]), (b, qbs[1])]
            continue
        if smode == "dual":
            for qb in range(4):
                inter = outproj_gen(*pending) if pending is not None else None
                sdpa_dual(b, qb, inter, pre=(pre0 if qb == 0 else 0))
                if inter is not None:
                    for _ in inter:
                        pass
                pending = (b, qb)
            continue
        if smode == "kco":
            kco_pass(b, 0, adv, None)
            kco_pass(b, 1, adv,
                     lambda qb, b=b: genq.append(outproj_gen(b, qb)))
            continue
        for qb in range(4):
            inter = outproj_gen(*pending) if pending is not None else None
            nk = 4 * (qb + 1)
            if spread2:
                ny = 16.0 / oq
                sdpa_block(b, qb, 0, inter=inter, per_kc=ny / (2 * nk))
                sdpa_block(b, qb, 1, inter=inter, per_kc=ny / (2 * nk))
            else:
                sdpa_block(b, qb, 0)
                sdpa_block(b, qb, 1, inter=inter, per_kc=16.0 / oq / nk)
            if inter is not None:
                for _ in inter:
                    pass
            pending = (b, qb)
    if smode == "kco":
        adv(10 ** 9)
    elif smode == "qpair":
        for p in pend2:
            for _ in outproj_gen(*p):
                pass
    else:
        for _ in outproj_gen(*pending):
            pass


def build_nc(reps=1, use_pool=True, out_bf16=True, dma_rot=True,
             spread2=True, pb=4, ob=6, sbufs=5, osplit=6, oq=2,
             lmode="mm1", lacc="dve2", vmode="dmat", ab=None, trim=True,
             pmode="quad", xb=None, strip_ldw=True, smode="qbo",
             rbf16=True, p2p=False, p2s=False, p2o=False, sb2=None,
             tb=4, nb=2, po=2, mmask=True, oeng="avv", vorder="smart",
             pre0=4, early_l=False, peng="aa", odma2=False):
    """Build the Bass module. reps>1 wraps the body in a For_i loop executing
    it that many times (used only for wall-clock timing measurements)."""
    import concourse.bass as bass
    import concourse.mybir as mybir
    import concourse.tile as tile

    dt = mybir.dt
    nc = bass.Bass("TRN2", target_bir_lowering=False, debug=False,
                   num_devices=N_CORES)

    xt_d = nc.dram_tensor("xt", [D, TOK], dt.bfloat16, kind="ExternalInput")
    wq_d = nc.dram_tensor("wq", [D, 256], dt.bfloat16, kind="ExternalInput")
    wk_d = nc.dram_tensor("wk", [D, 256], dt.bfloat16, kind="ExternalInput")
    wv_d = nc.dram_tensor("wv", [D, 256], dt.bfloat16, kind="ExternalInput")
    wo_d = nc.dram_tensor("wo", [256, D], dt.bfloat16, kind="ExternalInput")
    cs_d = nc.dram_tensor("cs", [128, T], dt.bfloat16, kind="ExternalInput")
    sn_d = nc.dram_tensor("sn", [128, T], dt.bfloat16, kind="ExternalInput")
    sns_d = nc.dram_tensor("sns", [128, T], dt.bfloat16, kind="ExternalInput")
    pt_d = nc.dram_tensor("pt", [128, 128], dt.bfloat16, kind="ExternalInput")
    onec_d = nc.dram_tensor("onec", [128, 1], dt.bfloat16, kind="ExternalInput")
    oner_d = nc.dram_tensor("oner", [1, 128], dt.bfloat16, kind="ExternalInput")
    msk_d = nc.dram_tensor("msk", [4, 128, 512], dt.bfloat16, kind="ExternalInput")
    mskt_d = nc.dram_tensor("mskt", [128, 128], dt.bfloat16, kind="ExternalInput")
    id_d = nc.dram_tensor("id", [128, 128], dt.bfloat16, kind="ExternalInput")
    out_d = nc.dram_tensor("out", [D, TOK],
                           dt.bfloat16 if out_bf16 else dt.float32,
                           kind="ExternalOutput")
    io = (xt_d, wq_d, wk_d, wv_d, wo_d, cs_d, sn_d, sns_d, pt_d, onec_d,
          oner_d, msk_d, mskt_d, id_d, out_d)

    with tile.TileContext(nc) as tc:
        import contextlib
        with contextlib.ExitStack() as ctx:
            consts = ctx.enter_context(tc.tile_pool(name="consts", bufs=1))
            xpool = ctx.enter_context(tc.tile_pool(
                name="xpool", bufs=(xb or {"single": 2, "pair": 3,
                                           "quad": 4}[pmode])))
            big = ctx.enter_context(tc.tile_pool(name="big", bufs=1))
            temps = ctx.enter_context(tc.tile_pool(name="temps", bufs=tb))
            ntemps = ctx.enter_context(tc.tile_pool(name="ntemps", bufs=nb))
            probsp = ctx.enter_context(tc.tile_pool(name="probs", bufs=pb))
            outp = ctx.enter_context(tc.tile_pool(name="outp", bufs=ob))
            if ab is None:
                ab = {"kco": 6, "dual": 4}.get(smode, 2)
            accp = (ctx.enter_context(tc.tile_pool(name="accp", bufs=ab))
                    if lmode != "mm" else None)
            n_po = 4 if smode == "kco" else po
            n_pl = 8 - 2 - sbufs if lmode == "mm" else 0
            # bank budget: n_po (psum_o) + n_pl + n_s1 ("s") + 2*n_s2 ("s2") = 8
            uses_s2 = p2p or p2s or p2o
            all_p2 = p2p and p2s and p2o
            if not uses_s2:
                n_s1 = sbufs if lmode == "mm" else 8 - n_po
                n_s2 = 0
            elif all_p2:
                n_s1, n_s2 = 0, (8 - n_po) // 2
            else:
                n_s2 = sb2 or 1
                n_s1 = 8 - n_po - n_pl - 2 * n_s2
            psum = (ctx.enter_context(tc.tile_pool(
                name="psum", bufs=n_s1, space="PSUM")) if n_s1 else None)
            psum2 = (ctx.enter_context(tc.tile_pool(
                name="psum2", bufs=n_s2, space="PSUM")) if n_s2 else None)
            psum_o = ctx.enter_context(tc.tile_pool(name="psum_o", bufs=n_po,
                                                    space="PSUM"))
            psum_l = (ctx.enter_context(tc.tile_pool(name="psum_l", bufs=n_pl,
                                                     space="PSUM"))
                      if n_pl else None)
            pools = (consts, xpool, big, temps, ntemps, probsp, outp, accp,
                     psum, psum2, psum_o, psum_l)
            kw = dict(use_pool=use_pool, out_bf16=out_bf16, dma_rot=dma_rot,
                      spread2=spread2, osplit=osplit, oq=oq,
                      lmode=lmode, lacc=lacc, vmode=vmode, trim=trim,
                      pmode=pmode, smode=smode, rbf16=rbf16,
                      p2p=p2p, p2s=p2s, p2o=p2o, mmask=mmask, oeng=oeng,
                      vorder=vorder, pre0=pre0, early_l=early_l, peng=peng,
                      odma2=odma2)
            if reps > 1:
                with tc.For_i(0, reps, 1):
                    _emit_body(nc, tc, pools, io, **kw)
            else:
                _emit_body(nc, tc, pools, io, **kw)

    if strip_ldw:
        _strip_redundant_ldw(nc)
    return nc


def _strip_redundant_ldw(nc):
    """Drop InstLdweights that reload the exact weights already resident in
    the PE array (same physical AP as the previous load in the block).
    Waits from dropped loads are merged into the next kept instruction."""
    import concourse.mybir as mybir

    n = 0
    for f in nc.m.functions:
        for bb in f.blocks:
            insts = bb.instructions
            new = []
            last_key = None
            pending_waits = []
            for inst in insts:
                if isinstance(inst, mybir.InstLdweights):
                    ap = inst.ins[0]
                    key = (str(ap.ap), ap.offset, ap.memref,
                           str(ap.dtype), str(inst.perf_mode),
                           str(inst.is_transpose),
                           str(getattr(inst, "tile_position", None)))
                    si = inst.sync_info
                    has_update = si is not None and si.on_update
                    if key == last_key and not has_update:
                        if si is not None and si.on_wait:
                            pending_waits.extend(si.on_wait)
                        n += 1
                        continue
                    last_key = key
                    new.append(inst)
                else:
                    if pending_waits and isinstance(inst, mybir.InstMatmult):
                        si = inst.sync_info
                        if si is None:
                            import bass_rust
                            inst.sync_info = bass_rust.SyncInfo(
                                on_wait=list(pending_waits), on_update=[])
                        else:
                            si.on_wait = list(si.on_wait) + pending_waits
                            inst.sync_info = si
                        pending_waits = []
                    new.append(inst)
            assert not pending_waits, "dropped LDW waits had no following matmul"
            bb.instructions = new
    return n


def _split_sync_waits(nc, max_waits=1):
    """Walrus in this env rejects instructions with too many sync waits.
    Hoist excess waits onto preceding same-engine nops."""
    import bass_rust
    import concourse.mybir as mybir

    n_split = 0
    for f in nc.m.functions:
        for bb in f.blocks:
            insts = bb.instructions
            new = []
            dirty = False
            for inst in insts:
                si = inst.sync_info
                if si is not None and si.on_wait and len(si.on_wait) > max_waits:
                    waits = list(si.on_wait)
                    for j, w in enumerate(waits[:-max_waits]):
                        n = mybir.InstNoOp(name=f"{inst.name}-wsplit{j}",
                                           ins=[], outs=[])
                        n.engine = inst.engine
                        n.sync_info = bass_rust.SyncInfo(on_wait=[w], on_update=[])
                        new.append(n)
                        n_split += 1
                    si.on_wait = waits[-max_waits:]
                    inst.sync_info = si
                    dirty = True
                new.append(inst)
            if dirty:
                bb.instructions = new
    return n_split


def _host_prep(x, cos, sin, Wqkv, Wout):
    """Shard + lay out inputs for each core. Returns in_maps list."""
    xf = np.ascontiguousarray(x.reshape(TOK, D).T).astype(BF16)        # [D, TOK]
    csT = np.ascontiguousarray(cos.T)                                   # [128, T]
    snT = np.ascontiguousarray(sin.T)
    cs4 = csT.astype(BF16)                                              # [128, T]
    sn4 = snT.astype(BF16)
    sns = np.concatenate([-snT[:64], snT[64:]], axis=0).astype(BF16)

    # rotate_half permutation: rot = P @ u ; pt = P.T
    P = np.zeros((128, 128), dtype=np.float32)
    P[np.arange(64), np.arange(64) + 64] = -1.0
    P[np.arange(64) + 64, np.arange(64)] = 1.0
    pt = np.ascontiguousarray(P.T).astype(BF16)

    onec = np.ones((128, 1), dtype=np.float32).astype(BF16)
    oner = np.ones((1, 128), dtype=np.float32).astype(BF16)

    msk = np.zeros((4, 128, 512), dtype=np.float32)
    for m in range(4):
        off = m * 128
        kk = np.arange(128)[:, None]
        qq = np.arange(512)[None, :]
        msk[m] = (off + kk <= qq).astype(np.float32)
    msk = msk.astype(BF16)

    # PE-side causal bias: psum += mskt.T @ id adds -1e5 above the diagonal
    kk = np.arange(128)[:, None]
    qq = np.arange(128)[None, :]
    mbias = np.where(kk <= qq, 0.0, -1e5).astype(np.float32)    # [k, q]
    mskt = np.ascontiguousarray(mbias.T).astype(BF16)
    ident = np.eye(128, dtype=np.float32).astype(BF16)

    in_maps = []
    for c in range(N_CORES):
        r0 = c * HPC * HD
        r1 = (c + 1) * HPC * HD
        wq = np.ascontiguousarray(Wqkv[r0:r1, :].T).astype(BF16)          # [D, 256]
        wk = np.ascontiguousarray(Wqkv[D + r0:D + r1, :].T).astype(BF16)
        wv = np.ascontiguousarray(Wqkv[2 * D + r0:2 * D + r1, :].T).astype(BF16)
        wo = np.ascontiguousarray(Wout[:, r0:r1].T).astype(BF16)          # [256, D]
        in_maps.append({
            "xt": xf, "wq": wq, "wk": wk, "wv": wv, "wo": wo,
            "cs": cs4, "sn": sn4, "sns": sns, "pt": pt, "onec": onec,
            "oner": oner,
            "msk": msk, "mskt": mskt, "id": ident,
        })
    return in_maps


def kernel(x, cos, sin, Wqkv, Wout):
    from concourse.bass_utils import run_bass_kernel_spmd

    x = np.asarray(x, dtype=np.float32)
    cos = np.asarray(cos, dtype=np.float32)
    sin = np.asarray(sin, dtype=np.float32)
    Wqkv = np.asarray(Wqkv, dtype=np.float32)
    Wout = np.asarray(Wout, dtype=np.float32)

    if "nc" not in _CACHE:
        nc = build_nc()
        _split_sync_waits(nc, max_waits=1)
        _CACHE["nc"] = nc
    nc = _CACHE["nc"]

    in_maps = _host_prep(x, cos, sin, Wqkv, Wout)
    res = run_bass_kernel_spmd(nc, in_maps, core_ids=list(range(N_CORES)))
    acc = np.zeros((D, TOK), dtype=np.float32)
    for c in range(N_CORES):
        acc += np.asarray(res.results[c]["out"], dtype=np.float32)
    return np.ascontiguousarray(acc.T).reshape(B, T, D)

